# revision 1
# baseline (speedup 1.0000x reference)
"""LoRA attention Bass kernel for 8x Trainium2 NeuronCores.

Sharding (Megatron tensor-parallel over heads):
  - Each of the 8 cores owns 2 heads (128 projection columns).
  - q/k/v projections column-sharded; out projection row-sharded;
    per-core partial outputs are summed on the host.
  - LoRA is merged into the base weights on the host (w_eff = w + a@u*scaling),
    which is exact up to fp32 rounding.

Device layout (per core):
  Phase 1: qT/kT computed transposed ([proj_col, seq]) straight off xT tiles;
           v computed in natural layout ([seq, proj_col]) from the same tiles.
  Phase 2: S^T = K @ Q^T per (batch, head) so softmax needs no transposes:
           exp on ACT (no max subtraction needed: scores ~ N(0,1)),
           P@V done as lhsT=[v | ones] so the softmax denominator falls out
           of the same matmul (row 64 of the PSUM output).
  Phase 3: out = attnout @ Wo_slice fused into the same loop, K=64 matmuls
           for the two head halves accumulated in PSUM.
"""

import os
import numpy as np

import concourse.bass as bass
import concourse.mybir as mybir
import concourse.tile as tile
from concourse import bacc
from concourse.bass_utils import run_bass_kernel_spmd

F32 = mybir.dt.float32
F32R = mybir.dt.float32r
AF = mybir.ActivationFunctionType

N_CORES = 8

# Full-problem dims (hardcoded per spec)
D_MODEL = 1024
N_HEADS = 16
D_K = 64
LORA_R = 8
SCALING = 2.0
B = 4
S = 2048


class Cfg:
    """Kernel build configuration (parameterized so tests can build small)."""

    def __init__(self, b=B, s=S, d=D_MODEL, cpc=128, dk=D_K, use_f32r=True,
                 bf16_stage1=False, bf16_attn=False):
        self.b = b                     # batches
        self.s = s                     # seq per batch
        self.d = d                     # model dim (contraction for projections)
        self.cpc = cpc                 # projection cols per core (2 heads x 64)
        self.dk = dk                   # head dim
        self.seq = b * s               # total rows
        self.nkc = d // 128            # k chunks for projections
        self.sc = 512                  # s-chunk width (free dim of matmuls)
        self.nsc = self.seq // self.sc  # s chunks over the whole input
        self.nt = s // 128             # t chunks per batch
        self.nsb = s // self.sc        # s chunks per batch
        self.use_f32r = use_f32r
        self.bf16_stage1 = bf16_stage1
        self.bf16_attn = bf16_attn


def _build_nc(cfg: Cfg):
    c = cfg
    nc = bacc.Bacc("TRN2", target_bir_lowering=False, debug=False,
                   num_devices=N_CORES)

    mmdt = F32R if c.use_f32r else F32
    MMD = mmdt  # dtype for matmul-feeding tensors end-to-end

    def r(ap):
        return ap.bitcast(mmdt)

    xT = nc.dram_tensor("xT", [c.d, c.seq], MMD, kind="ExternalInput").ap()
    wq = nc.dram_tensor("wq", [c.d, c.cpc], MMD, kind="ExternalInput").ap()
    wk = nc.dram_tensor("wk", [c.d, c.cpc], MMD, kind="ExternalInput").ap()
    wv = nc.dram_tensor("wv", [c.d, c.cpc], MMD, kind="ExternalInput").ap()
    wo = nc.dram_tensor("wo", [c.cpc, c.d], MMD, kind="ExternalInput").ap()
    bq = nc.dram_tensor("bq", [c.cpc, 1], F32, kind="ExternalInput").ap()
    bk = nc.dram_tensor("bk", [c.cpc, 1], F32, kind="ExternalInput").ap()
    out = nc.dram_tensor("out", [c.seq, c.d], F32, kind="ExternalOutput").ap()

    dk = c.dk
    n_tchunks = c.seq // 128  # global 128-row seq chunks

    with tile.TileContext(nc) as tc:
        with tc.tile_pool(name="persist", bufs=1) as persist:
            # Persistent SBUF tensors
            qT_sb = persist.tile([128, c.seq], MMD, tag="qT")
            kT_sb = persist.tile([128, c.seq], MMD, tag="kT")
            # v natural + ones columns: [.., 0:64]=headA, 64=ones, 65:129=headB, 129=ones
            v_sb = persist.tile([128, n_tchunks, 2 * dk + 2], MMD, tag="v")
            wq_sb = persist.tile([128, c.nkc, c.cpc], MMD, tag="wq")
            wk_sb = persist.tile([128, c.nkc, c.cpc], MMD, tag="wk")
            wv_sb = persist.tile([128, c.nkc, c.cpc], MMD, tag="wv")
            woA_sb = persist.tile([dk, c.d], MMD, tag="woA")
            woB_sb = persist.tile([dk, c.d], MMD, tag="woB")
            bq_sb = persist.tile([c.cpc, 1], F32, tag="bq")
            bk_sb = persist.tile([c.cpc, 1], F32, tag="bk")

            nc.sync.dma_start(out=wq_sb[:], in_=wq.rearrange("(kc p) m -> p kc m", p=128))
            nc.sync.dma_start(out=wk_sb[:], in_=wk.rearrange("(kc p) m -> p kc m", p=128))
            nc.sync.dma_start(out=wv_sb[:], in_=wv.rearrange("(kc p) m -> p kc m", p=128))
            nc.sync.dma_start(out=woA_sb[:], in_=wo[0:dk, :])
            nc.sync.dma_start(out=woB_sb[:], in_=wo[dk:2 * dk, :])
            nc.sync.dma_start(out=bq_sb[:], in_=bq[:])
            nc.sync.dma_start(out=bk_sb[:], in_=bk[:])

            # ones columns for the fused softmax denominator (memset cannot
            # write f32r, so memset an F32 scratch and broadcast-copy)
            ones_f32 = persist.tile([128, 1], F32, tag="ones_f32")
            nc.vector.memset(ones_f32[:], 1.0)
            nc.vector.tensor_copy(
                v_sb[:, :, dk:dk + 1],
                ones_f32[:].unsqueeze(1).to_broadcast([128, n_tchunks, 1]))
            nc.vector.tensor_copy(
                v_sb[:, :, 2 * dk + 1:2 * dk + 2],
                ones_f32[:].unsqueeze(1).to_broadcast([128, n_tchunks, 1]))

            # ones row at partition dk, used to broadcast the recip row
            ones_sb = persist.tile([dk + 1, dk], MMD, tag="ones")
            nc.vector.tensor_copy(
                ones_sb[:], ones_f32[0:dk + 1, :].to_broadcast([dk + 1, dk]))

            # ---------------- Phase 1: projections ----------------
            nj = c.sc // 128
            with tc.tile_pool(name="xin", bufs=3) as xpool, \
                 tc.tile_pool(name="p1ps", bufs=2, space="PSUM") as p1ps, \
                 tc.tile_pool(name="vpps", bufs=nj, space="PSUM") as vpps:
                for sc_i in range(c.nsc):
                    s0 = sc_i * c.sc
                    q_ps = p1ps.tile([128, c.sc], F32, tag="q")
                    k_ps = p1ps.tile([128, c.sc], F32, tag="k")
                    v_ps = [vpps.tile([128, 128], F32, tag="vp",
                                      name=f"v_ps_{sc_i}_{j}")
                            for j in range(nj)]
                    for kc in range(c.nkc):
                        x_t = xpool.tile([128, c.sc], MMD, tag="x")
                        nc.sync.dma_start(
                            out=x_t[:],
                            in_=xT[kc * 128:(kc + 1) * 128, s0:s0 + c.sc])
                        st = (kc == 0)
                        sp = (kc == c.nkc - 1)
                        nc.tensor.matmul(q_ps[:], r(wq_sb[:, kc, :]), r(x_t[:]),
                                         start=st, stop=sp)
                        nc.tensor.matmul(k_ps[:], r(wk_sb[:, kc, :]), r(x_t[:]),
                                         start=st, stop=sp)
                        for j in range(nj):
                            nc.tensor.matmul(
                                v_ps[j][:],
                                r(x_t[:, j * 128:(j + 1) * 128]),
                                r(wv_sb[:, kc, :]),
                                start=st, stop=sp)
                    nc.scalar.activation(qT_sb[:, s0:s0 + c.sc], q_ps[:],
                                         AF.Identity, bias=bq_sb[:])
                    nc.scalar.activation(kT_sb[:, s0:s0 + c.sc], k_ps[:],
                                         AF.Identity, bias=bk_sb[:])
                    tc0 = sc_i * nj
                    for j in range(nj):
                        nc.vector.tensor_copy(
                            v_sb[:, tc0 + j, 0:dk], v_ps[j][:, 0:dk])
                        nc.vector.tensor_copy(
                            v_sb[:, tc0 + j, dk + 1:2 * dk + 1],
                            v_ps[j][:, dk:2 * dk])

            # ---------------- Phase 2+3: attention + out-proj ----------------
            with tc.tile_pool(name="sps", bufs=2, space="PSUM") as spool, \
                 tc.tile_pool(name="pvps", bufs=1, space="PSUM") as pvpool, \
                 tc.tile_pool(name="ops", bufs=2, space="PSUM") as opool, \
                 tc.tile_pool(name="exp", bufs=4) as epool, \
                 tc.tile_pool(name="norm", bufs=2) as npool, \
                 tc.tile_pool(name="bc", bufs=2, space="PSUM") as bcpool, \
                 tc.tile_pool(name="bcs", bufs=2) as bcspool, \
                 tc.tile_pool(name="rec", bufs=2) as rpool, \
                 tc.tile_pool(name="osb", bufs=2) as osbpool:
                for b_i in range(c.b):
                    for sb_i in range(c.nsb):
                        s0 = b_i * c.s + sb_i * c.sc
                        pv_a = pvpool.tile([dk + 1, c.sc], F32, tag="pva")
                        pv_b = pvpool.tile([dk + 1, c.sc], F32, tag="pvb")
                        for t in range(c.nt):
                            t0 = b_i * c.s + t * 128
                            tci = b_i * c.nt + t
                            s_a = spool.tile([128, c.sc], F32, tag="s")
                            nc.tensor.matmul(
                                s_a[:], r(kT_sb[0:dk, t0:t0 + 128]),
                                r(qT_sb[0:dk, s0:s0 + c.sc]),
                                start=True, stop=True)
                            e_a = epool.tile([128, c.sc], MMD, tag="e")
                            nc.scalar.activation(e_a[:], s_a[:], AF.Exp,
                                                 scale=1.0 / np.sqrt(dk))
                            nc.tensor.matmul(
                                pv_a[:], r(v_sb[:, tci, 0:dk + 1]), r(e_a[:]),
                                start=(t == 0), stop=(t == c.nt - 1))
                            s_b = spool.tile([128, c.sc], F32, tag="s")
                            nc.tensor.matmul(
                                s_b[:], r(kT_sb[dk:2 * dk, t0:t0 + 128]),
                                r(qT_sb[dk:2 * dk, s0:s0 + c.sc]),
                                start=True, stop=True)
                            e_b = epool.tile([128, c.sc], MMD, tag="e")
                            nc.scalar.activation(e_b[:], s_b[:], AF.Exp,
                                                 scale=1.0 / np.sqrt(dk))
                            nc.tensor.matmul(
                                pv_b[:], r(v_sb[:, tci, dk + 1:2 * dk + 2]),
                                r(e_b[:]),
                                start=(t == 0), stop=(t == c.nt - 1))

                        # normalize: out_norm = out_unnorm * (1/denom), denom
                        # is row dk of the PV accumulators
                        rec_a = rpool.tile([dk + 1, c.sc], MMD, tag="ra")
                        rec_b = rpool.tile([dk + 1, c.sc], MMD, tag="rb")
                        with nc.allow_low_precision(
                                reason="recip rounded to tf32 for f32r matmul"):
                            nc.vector.reciprocal(rec_a[dk:dk + 1, :],
                                                 pv_a[dk:dk + 1, :])
                            nc.vector.reciprocal(rec_b[dk:dk + 1, :],
                                                 pv_b[dk:dk + 1, :])
                        # broadcast recip row (partition dk) to partitions
                        # 0..dk-1 via a K=1 matmul with a ones row
                        bc_a = bcpool.tile([dk, c.sc], F32, tag="bc")
                        bc_b = bcpool.tile([dk, c.sc], F32, tag="bc")
                        nc.tensor.matmul(
                            bc_a[:], r(ones_sb[dk:dk + 1, :]),
                            r(rec_a[dk:dk + 1, :]), start=True, stop=True)
                        nc.tensor.matmul(
                            bc_b[:], r(ones_sb[dk:dk + 1, :]),
                            r(rec_b[dk:dk + 1, :]), start=True, stop=True)
                        bcs_a = bcspool.tile([dk, c.sc], F32, tag="bcs")
                        bcs_b = bcspool.tile([dk, c.sc], F32, tag="bcs")
                        nc.vector.tensor_copy(bcs_a[:], bc_a[:])
                        nc.vector.tensor_copy(bcs_b[:], bc_b[:])
                        norm_a = npool.tile([dk, c.sc], MMD, tag="na")
                        norm_b = npool.tile([dk, c.sc], MMD, tag="nb")
                        nc.vector.tensor_tensor(
                            norm_a[:], pv_a[0:dk, :], bcs_a[:],
                            mybir.AluOpType.mult)
                        nc.vector.tensor_tensor(
                            norm_b[:], pv_b[0:dk, :], bcs_b[:],
                            mybir.AluOpType.mult)

                        # fused out-projection for this s-chunk
                        ew = min(512, c.d)
                        for j in range(c.sc // 128):
                            o_t = osbpool.tile([128, c.d], F32, tag="osb")
                            for e in range(c.d // ew):
                                o_ps = opool.tile([128, ew], F32, tag="o")
                                nc.tensor.matmul(
                                    o_ps[:],
                                    r(norm_a[:, j * 128:(j + 1) * 128]),
                                    r(woA_sb[:, e * ew:(e + 1) * ew]),
                                    start=True, stop=False)
                                nc.tensor.matmul(
                                    o_ps[:],
                                    r(norm_b[:, j * 128:(j + 1) * 128]),
                                    r(woB_sb[:, e * ew:(e + 1) * ew]),
                                    start=False, stop=True)
                                nc.vector.tensor_copy(
                                    o_t[:, e * ew:(e + 1) * ew], o_ps[:])
                            nc.sync.dma_start(
                                out=out[s0 + j * 128:s0 + (j + 1) * 128, :],
                                in_=o_t[:])

    nc.compile()
    return nc


_NC_CACHE = {}


def get_nc(cfg: Cfg | None = None):
    cfg = cfg or Cfg()
    key = (cfg.b, cfg.s, cfg.d, cfg.cpc, cfg.dk, cfg.use_f32r,
           cfg.bf16_stage1, cfg.bf16_attn)
    if key not in _NC_CACHE:
        _NC_CACHE[key] = _build_nc(cfg)
    return _NC_CACHE[key]


def kernel(x, w_q, b_q, w_k, b_k, w_v, b_v, w_o, b_o,
           a_q, u_q, a_k, u_k, a_v, u_v):
    cfg = Cfg()
    c = cfg
    x = np.asarray(x, np.float32)
    w_q = np.asarray(w_q, np.float32)
    w_k = np.asarray(w_k, np.float32)
    w_v = np.asarray(w_v, np.float32)
    w_o = np.asarray(w_o, np.float32)
    b_q = np.asarray(b_q, np.float32)
    b_k = np.asarray(b_k, np.float32)
    b_v = np.asarray(b_v, np.float32)
    b_o = np.asarray(b_o, np.float32)

    def merge(w, a, u):
        return (w.astype(np.float64)
                + (np.asarray(a, np.float64) @ np.asarray(u, np.float64))
                * SCALING).astype(np.float32)

    wq_eff = merge(w_q, a_q, u_q)
    wk_eff = merge(w_k, a_k, u_k)
    wv_eff = merge(w_v, a_v, u_v)

    xT = np.ascontiguousarray(x.reshape(c.seq, c.d).T)
    in_maps = []
    for i in range(N_CORES):
        sl = slice(i * c.cpc, (i + 1) * c.cpc)
        in_maps.append({
            "xT": xT,
            "wq": np.ascontiguousarray(wq_eff[:, sl]),
            "wk": np.ascontiguousarray(wk_eff[:, sl]),
            "wv": np.ascontiguousarray(wv_eff[:, sl]),
            "wo": np.ascontiguousarray(w_o[sl, :]),
            "bq": np.ascontiguousarray(b_q[sl]).reshape(c.cpc, 1),
            "bk": np.ascontiguousarray(b_k[sl]).reshape(c.cpc, 1),
        })

    nc = get_nc(cfg)
    res = run_bass_kernel_spmd(nc, in_maps, list(range(N_CORES)))
    out = np.zeros((c.seq, c.d), np.float32)
    for i in range(N_CORES):
        out += res.results[i]["out"]
    # v-bias rides through softmax as a constant row; b_o is plain bias
    out += (b_v @ w_o + b_o).astype(np.float32)
    return out.reshape(B, S, D_MODEL).astype(np.float32)



# revision 6
# speedup vs baseline: 1.4244x; 1.4244x over previous
"""LoRA attention Bass kernel for 8x Trainium2 NeuronCores.

Sharding (Megatron tensor-parallel over heads):
  - Each of the 8 cores owns 2 heads (128 projection columns).
  - q/k/v projections column-sharded; out projection row-sharded;
    per-core partial outputs are summed on the host.
  - LoRA is merged into the base weights on the host (w_eff = w + a@u*scaling),
    which is exact up to fp32 rounding.

All matmul operands are bf16 (fp32/f32r matmuls trip the PE power throttle
to a 50% duty cycle; bf16 streams at 1 row/cycle at 2.4 GHz). PSUM
accumulation stays fp32.

Device schedule (per core):
  Phase 1 (proj): qT/kT computed transposed ([proj_col, seq]) from xT tiles;
      v computed in natural layout ([seq, proj_col]). Biases folded in via
      DVE tensor_scalar on the PSUM->SBUF copy. Projections for batch b+1 are
      interleaved into batch b's attention t-loop to fill PE idle slots
      (the t-loop is ACT-bound).
  Phase 2 (attention): S^T = K @ Q^T per (batch, head); scores for two
      t-chunks share one 2-bank PSUM tile so a single ACT exp instruction
      covers 1024 elements (halves ACT per-instruction overhead). P@V uses
      lhsT=[v | ones] so the softmax denominator falls out of the same
      matmul (row 64 of the PSUM output).
  Phase 3 (out-proj): recip via DVE reciprocal_approx_fast, broadcast to 64
      partitions via a K=1 ones matmul, normalize+cast to bf16 on DVE, then
      out = attnout @ Wo_slice accumulated in PSUM and DMA'd out.

PSUM budget (8 banks): s2 tag 2x[128,1024] (scores / bcast / out-proj) = 4,
  pv_a + pv_b = 2, proj accumulators (q/k/v rotating, bufs=2) = 2.
"""

import numpy as np
import ml_dtypes

import concourse.bass as bass
import concourse.mybir as mybir
import concourse.tile as tile
from concourse import bacc
from concourse.bass_utils import run_bass_kernel_spmd

F32 = mybir.dt.float32
F32R = mybir.dt.float32r
BF16 = mybir.dt.bfloat16
AF = mybir.ActivationFunctionType
ALU = mybir.AluOpType

N_CORES = 8

# Full-problem dims (hardcoded per spec)
D_MODEL = 1024
N_HEADS = 16
D_K = 64
LORA_R = 8
SCALING = 2.0
B = 4
S = 2048

NP_BF16 = ml_dtypes.bfloat16


class Cfg:
    def __init__(self, b=B, s=S, d=D_MODEL, cpc=128, dk=D_K):
        self.b = b                      # batches
        self.s = s                      # seq per batch
        self.d = d                      # model dim
        self.cpc = cpc                  # projection cols per core (2 heads x 64)
        self.dk = dk                    # head dim
        self.seq = b * s                # total rows
        self.nkc = d // 128             # k chunks for projections
        self.sc = 512                   # s-chunk width (free dim of matmuls)
        self.nsc = self.seq // self.sc  # s chunks over the whole input
        self.ntb = s // 128             # t chunks per batch
        self.ntp = self.ntb // 2        # t-chunk pairs per batch
        self.nsb = s // self.sc         # s chunks per batch
        self.ntc_g = self.seq // 128    # global t chunks


def _build_nc(cfg: Cfg):
    c = cfg
    dk = c.dk
    nc = bacc.Bacc("TRN2", target_bir_lowering=False, debug=False,
                   num_devices=N_CORES)

    xT = nc.dram_tensor("xT", [c.d, c.seq], BF16, kind="ExternalInput").ap()
    wq = nc.dram_tensor("wq", [c.d, c.cpc], BF16, kind="ExternalInput").ap()
    wk = nc.dram_tensor("wk", [c.d, c.cpc], BF16, kind="ExternalInput").ap()
    wv = nc.dram_tensor("wv", [c.d, c.cpc], BF16, kind="ExternalInput").ap()
    wo = nc.dram_tensor("wo", [c.cpc, c.d], BF16, kind="ExternalInput").ap()
    bq = nc.dram_tensor("bq", [c.cpc, 1], F32, kind="ExternalInput").ap()
    bk = nc.dram_tensor("bk", [c.cpc, 1], F32, kind="ExternalInput").ap()
    out = nc.dram_tensor("out", [c.seq, c.d], F32, kind="ExternalOutput").ap()

    xT_r = xT.rearrange("(kc p) s -> p kc s", p=128)

    with tile.TileContext(nc) as tc:
        with tc.tile_pool(name="persist", bufs=1) as persist:
            qT_sb = persist.tile([128, c.seq], BF16, tag="qT")
            kT_sb = persist.tile([128, c.seq], BF16, tag="kT")
            # v natural + ones cols: [0:64]=headA, 64=ones, 65:129=headB, 129=ones
            v_sb = persist.tile([128, c.ntc_g, 2 * dk + 2], BF16, tag="v")
            wq_sb = persist.tile([128, c.nkc, c.cpc], BF16, tag="wq")
            wk_sb = persist.tile([128, c.nkc, c.cpc], BF16, tag="wk")
            wv_sb = persist.tile([128, c.nkc, c.cpc], BF16, tag="wv")
            woA_sb = persist.tile([dk, c.d], BF16, tag="woA")
            woB_sb = persist.tile([dk, c.d], BF16, tag="woB")
            bq_sb = persist.tile([c.cpc, 1], F32, tag="bq")
            bk_sb = persist.tile([c.cpc, 1], F32, tag="bk")
            ones64 = persist.tile([1, dk], BF16, tag="ones64")

            nc.sync.dma_start(out=wq_sb[:], in_=wq.rearrange("(kc p) m -> p kc m", p=128))
            nc.sync.dma_start(out=wk_sb[:], in_=wk.rearrange("(kc p) m -> p kc m", p=128))
            nc.sync.dma_start(out=wv_sb[:], in_=wv.rearrange("(kc p) m -> p kc m", p=128))
            nc.sync.dma_start(out=woA_sb[:], in_=wo[0:dk, :])
            nc.sync.dma_start(out=woB_sb[:], in_=wo[dk:2 * dk, :])
            nc.sync.dma_start(out=bq_sb[:], in_=bq[:])
            nc.sync.dma_start(out=bk_sb[:], in_=bk[:])

            ones_f32 = persist.tile([128, 1], F32, tag="ones_f32")
            nc.vector.memset(ones_f32[:], 1.0)
            nc.vector.tensor_copy(ones64[:], ones_f32[0:1, :].to_broadcast([1, dk]))
            nc.vector.tensor_copy(
                v_sb[:, :, dk:dk + 1],
                ones_f32[:].unsqueeze(1).to_broadcast([128, c.ntc_g, 1]))
            nc.vector.tensor_copy(
                v_sb[:, :, 2 * dk + 1:2 * dk + 2],
                ones_f32[:].unsqueeze(1).to_broadcast([128, c.ntc_g, 1]))

            with tc.tile_pool(name="xin", bufs=2) as xpool, \
                 tc.tile_pool(name="ps2", bufs=2, space="PSUM") as ps2, \
                 tc.tile_pool(name="pvp", bufs=1, space="PSUM") as pvp, \
                 tc.tile_pool(name="prj", bufs=2, space="PSUM") as prj, \
                 tc.tile_pool(name="exp", bufs=3) as epool, \
                 tc.tile_pool(name="norm", bufs=2) as npool, \
                 tc.tile_pool(name="rec", bufs=2) as rpool, \
                 tc.tile_pool(name="osb", bufs=3) as opool:

                xc_tiles = {}

                def dma_x(sc_i):
                    x_t = xpool.tile([128, c.nkc, c.sc], BF16, tag="x",
                                     name=f"xc_{sc_i}")
                    s0 = sc_i * c.sc
                    nc.sync.dma_start(out=x_t[:], in_=xT_r[:, :, s0:s0 + c.sc])
                    xc_tiles[sc_i] = x_t

                # Per-chunk projection state (psum tiles held across stages)
                pstate = {}

                def proj_stage(sc_i, stage):
                    """Emit 1/8th of projection chunk sc_i (stages 0..7)."""
                    xc = xc_tiles[sc_i]
                    s0 = sc_i * c.sc
                    st = pstate.setdefault(sc_i, {})
                    if stage == 0:
                        q_ps = prj.tile([128, c.sc], F32, tag="prj",
                                        name=f"q_ps_{sc_i}")
                        st["q"] = q_ps
                        for kc in range(4):
                            nc.tensor.matmul(q_ps[:], wq_sb[:, kc, :],
                                             xc[:, kc, :],
                                             start=(kc == 0), stop=False)
                    elif stage == 1:
                        q_ps = st.pop("q")
                        for kc in range(4, c.nkc):
                            nc.tensor.matmul(q_ps[:], wq_sb[:, kc, :],
                                             xc[:, kc, :],
                                             start=False, stop=(kc == c.nkc - 1))
                        nc.vector.tensor_scalar(
                            qT_sb[:, s0:s0 + c.sc], q_ps[:], bq_sb[:], None,
                            ALU.add)
                    elif stage == 2:
                        k_ps = prj.tile([128, c.sc], F32, tag="prj",
                                        name=f"k_ps_{sc_i}")
                        st["k"] = k_ps
                        for kc in range(4):
                            nc.tensor.matmul(k_ps[:], wk_sb[:, kc, :],
                                             xc[:, kc, :],
                                             start=(kc == 0), stop=False)
                    elif stage == 3:
                        k_ps = st.pop("k")
                        for kc in range(4, c.nkc):
                            nc.tensor.matmul(k_ps[:], wk_sb[:, kc, :],
                                             xc[:, kc, :],
                                             start=False, stop=(kc == c.nkc - 1))
                        nc.vector.tensor_scalar(
                            kT_sb[:, s0:s0 + c.sc], k_ps[:], bk_sb[:], None,
                            ALU.add)
                    elif stage in (4, 5, 6):
                        if stage == 4:
                            v_ps = prj.tile([128, 4, 128], F32, tag="prj",
                                            name=f"v_ps_{sc_i}")
                            st["v"] = v_ps
                        v_ps = st["v"]
                        # One accumulation group for the whole bank: start
                        # zeroes the full 2KB zero region, so only the very
                        # first matmul may set start and only the last stop.
                        for kc in range(2 * (stage - 4), 2 * (stage - 4) + 2):
                            for j in range(4):
                                nc.tensor.matmul(
                                    v_ps[:, j, :],
                                    xc[:, kc, j * 128:(j + 1) * 128],
                                    wv_sb[:, kc, :],
                                    start=(kc == 0 and j == 0), stop=False,
                                    skip_group_check=True)
                    else:  # stage 7
                        v_ps = st.pop("v")
                        for kc in (6, 7):
                            for j in range(4):
                                nc.tensor.matmul(
                                    v_ps[:, j, :],
                                    xc[:, kc, j * 128:(j + 1) * 128],
                                    wv_sb[:, kc, :],
                                    start=False, stop=(kc == 7 and j == 3),
                                    skip_group_check=True)
                        tc0 = sc_i * 4
                        for j in range(4):
                            nc.vector.tensor_copy(
                                v_sb[:, tc0 + j, 0:dk], v_ps[:, j, 0:dk])
                            nc.vector.tensor_copy(
                                v_sb[:, tc0 + j, dk + 1:2 * dk + 1],
                                v_ps[:, j, dk:2 * dk])
                        del xc_tiles[sc_i]
                        pstate.pop(sc_i, None)

                # ---------------- batch 0 projections upfront ----------------
                for sc_i in range(c.nsb):
                    dma_x(sc_i)
                for sc_i in range(c.nsb):
                    for stage in range(8):
                        proj_stage(sc_i, stage)
                # prefetch x for the first interleaved proj chunk
                if c.b > 1:
                    dma_x(c.nsb)

                # ---------------- main loop ----------------
                for b_i in range(c.b):
                    for sb in range(c.nsb):
                        s0 = b_i * c.s + sb * c.sc
                        proj_c = (b_i + 1) * c.nsb + sb if b_i + 1 < c.b else None
                        pva = pvp.tile([dk + 1, c.sc], F32, tag="pva")
                        pvb = pvp.tile([dk + 1, c.sc], F32, tag="pvb")
                        for tp in range(c.ntp):
                            t0 = b_i * c.s + (2 * tp) * 128
                            t1 = t0 + 128
                            s2a = ps2.tile([128, 2 * c.sc], F32, tag="s2",
                                           name=f"s2a_{b_i}_{sb}_{tp}")
                            nc.tensor.matmul(
                                s2a[:, 0:c.sc], kT_sb[0:dk, t0:t0 + 128],
                                qT_sb[0:dk, s0:s0 + c.sc],
                                start=True, stop=True)
                            nc.tensor.matmul(
                                s2a[:, c.sc:2 * c.sc], kT_sb[0:dk, t1:t1 + 128],
                                qT_sb[0:dk, s0:s0 + c.sc],
                                start=True, stop=True)
                            e2a = epool.tile([128, 2 * c.sc], BF16, tag="e2",
                                             name=f"e2a_{b_i}_{sb}_{tp}")
                            nc.scalar.activation(e2a[:], s2a[:], AF.Exp,
                                                 scale=1.0 / np.sqrt(dk))
                            s2b = ps2.tile([128, 2 * c.sc], F32, tag="s2",
                                           name=f"s2b_{b_i}_{sb}_{tp}")
                            nc.tensor.matmul(
                                s2b[:, 0:c.sc], kT_sb[dk:2 * dk, t0:t0 + 128],
                                qT_sb[dk:2 * dk, s0:s0 + c.sc],
                                start=True, stop=True)
                            nc.tensor.matmul(
                                s2b[:, c.sc:2 * c.sc], kT_sb[dk:2 * dk, t1:t1 + 128],
                                qT_sb[dk:2 * dk, s0:s0 + c.sc],
                                start=True, stop=True)
                            e2b = epool.tile([128, 2 * c.sc], BF16, tag="e2",
                                             name=f"e2b_{b_i}_{sb}_{tp}")
                            nc.scalar.activation(e2b[:], s2b[:], AF.Exp,
                                                 scale=1.0 / np.sqrt(dk))
                            tca = b_i * c.ntb + 2 * tp
                            tcb = tca + 1
                            nc.tensor.matmul(
                                pva[:], v_sb[:, tca, 0:dk + 1], e2a[:, 0:c.sc],
                                start=(tp == 0), stop=False)
                            nc.tensor.matmul(
                                pva[:], v_sb[:, tcb, 0:dk + 1],
                                e2a[:, c.sc:2 * c.sc],
                                start=False, stop=(tp == c.ntp - 1))
                            nc.tensor.matmul(
                                pvb[:], v_sb[:, tca, dk + 1:2 * dk + 2],
                                e2b[:, 0:c.sc],
                                start=(tp == 0), stop=False)
                            nc.tensor.matmul(
                                pvb[:], v_sb[:, tcb, dk + 1:2 * dk + 2],
                                e2b[:, c.sc:2 * c.sc],
                                start=False, stop=(tp == c.ntp - 1))
                            if proj_c is not None:
                                for st_i in range(tp * 8 // c.ntp,
                                                  (tp + 1) * 8 // c.ntp):
                                    proj_stage(proj_c, st_i)

                        # ---- normalize ----
                        rec_af = rpool.tile([1, c.sc], F32, tag="recf")
                        rec_bf = rpool.tile([1, c.sc], F32, tag="recf")
                        with nc.allow_low_precision(
                                reason="softmax denom reciprocal"):
                            nc.vector.reciprocal(
                                rec_af[:], pva[dk:dk + 1, :])
                            nc.vector.reciprocal(
                                rec_bf[:], pvb[dk:dk + 1, :])
                        rec_ab = rpool.tile([1, c.sc], BF16, tag="recb")
                        rec_bb = rpool.tile([1, c.sc], BF16, tag="recb")
                        nc.vector.tensor_copy(rec_ab[:], rec_af[:])
                        nc.vector.tensor_copy(rec_bb[:], rec_bf[:])
                        bc2 = ps2.tile([128, 2 * c.sc], F32, tag="s2",
                                       name=f"bc2_{b_i}_{sb}")
                        nc.tensor.matmul(bc2[0:dk, 0:c.sc], ones64[:],
                                         rec_ab[:], start=True, stop=True)
                        nc.tensor.matmul(bc2[0:dk, c.sc:2 * c.sc], ones64[:],
                                         rec_bb[:], start=True, stop=True)
                        bcs = npool.tile([dk, 2 * c.sc], F32, tag="bcs")
                        nc.vector.tensor_copy(bcs[:], bc2[0:dk, :])
                        norm_a = npool.tile([dk, c.sc], BF16, tag="na")
                        norm_b = npool.tile([dk, c.sc], BF16, tag="nb")
                        nc.vector.tensor_tensor(
                            norm_a[:], pva[0:dk, :], bcs[:, 0:c.sc],
                            ALU.mult)
                        nc.vector.tensor_tensor(
                            norm_b[:], pvb[0:dk, :], bcs[:, c.sc:2 * c.sc],
                            ALU.mult)

                        # ---- out projection ----
                        for j in range(c.sc // 128):
                            o2 = ps2.tile([128, 2 * c.sc], F32, tag="s2",
                                          name=f"o2_{b_i}_{sb}_{j}")
                            for e in range(2):
                                nc.tensor.matmul(
                                    o2[:, e * c.sc:(e + 1) * c.sc],
                                    norm_a[:, j * 128:(j + 1) * 128],
                                    woA_sb[:, e * c.sc:(e + 1) * c.sc],
                                    start=True, stop=False)
                                nc.tensor.matmul(
                                    o2[:, e * c.sc:(e + 1) * c.sc],
                                    norm_b[:, j * 128:(j + 1) * 128],
                                    woB_sb[:, e * c.sc:(e + 1) * c.sc],
                                    start=False, stop=True)
                            osb = opool.tile([128, c.d], F32, tag="osb",
                                             name=f"osb_{b_i}_{sb}_{j}")
                            nc.vector.tensor_copy(osb[:], o2[:])
                            nc.sync.dma_start(
                                out=out[s0 + j * 128:s0 + (j + 1) * 128, :],
                                in_=osb[:])

                        # prefetch x for the next interleaved proj chunk
                        if proj_c is not None:
                            nxt = proj_c + 1
                            if nxt < c.nsc and nxt not in xc_tiles:
                                dma_x(nxt)

    nc.compile()
    return nc


_NC_CACHE = {}


def get_nc(cfg: Cfg | None = None):
    cfg = cfg or Cfg()
    key = (cfg.b, cfg.s, cfg.d, cfg.cpc, cfg.dk)
    if key not in _NC_CACHE:
        _NC_CACHE[key] = _build_nc(cfg)
    return _NC_CACHE[key]


def make_in_maps(x, w_q, b_q, w_k, b_k, w_v, b_v, w_o, b_o,
                 a_q, u_q, a_k, u_k, a_v, u_v, cfg: Cfg | None = None):
    """Host-side prep: merge LoRA, transpose x, cast to bf16, shard."""
    c = cfg or Cfg()
    x = np.asarray(x, np.float32)
    w_o = np.asarray(w_o, np.float32)

    def merge(w, a, u):
        return (np.asarray(w, np.float64)
                + (np.asarray(a, np.float64) @ np.asarray(u, np.float64))
                * SCALING).astype(np.float32)

    wq_eff = merge(w_q, a_q, u_q)
    wk_eff = merge(w_k, a_k, u_k)
    wv_eff = merge(w_v, a_v, u_v)

    xT = np.ascontiguousarray(x.reshape(c.seq, c.d).T).astype(NP_BF16)
    b_q = np.asarray(b_q, np.float32)
    b_k = np.asarray(b_k, np.float32)
    in_maps = []
    for i in range(N_CORES):
        sl = slice(i * c.cpc, (i + 1) * c.cpc)
        in_maps.append({
            "xT": xT,
            "wq": np.ascontiguousarray(wq_eff[:, sl]).astype(NP_BF16),
            "wk": np.ascontiguousarray(wk_eff[:, sl]).astype(NP_BF16),
            "wv": np.ascontiguousarray(wv_eff[:, sl]).astype(NP_BF16),
            "wo": np.ascontiguousarray(w_o[sl, :]).astype(NP_BF16),
            "bq": np.ascontiguousarray(b_q[sl]).reshape(c.cpc, 1),
            "bk": np.ascontiguousarray(b_k[sl]).reshape(c.cpc, 1),
        })
    return in_maps


def kernel(x, w_q, b_q, w_k, b_k, w_v, b_v, w_o, b_o,
           a_q, u_q, a_k, u_k, a_v, u_v):
    cfg = Cfg()
    c = cfg
    in_maps = make_in_maps(x, w_q, b_q, w_k, b_k, w_v, b_v, w_o, b_o,
                           a_q, u_q, a_k, u_k, a_v, u_v, cfg)
    nc = get_nc(cfg)
    res = run_bass_kernel_spmd(nc, in_maps, list(range(N_CORES)))
    out = np.zeros((c.seq, c.d), np.float32)
    for i in range(N_CORES):
        out += res.results[i]["out"]
    # v-bias rides through softmax as a constant row; b_o is plain bias
    b_v = np.asarray(b_v, np.float32)
    b_o = np.asarray(b_o, np.float32)
    w_o = np.asarray(w_o, np.float32)
    out += (b_v @ w_o + b_o).astype(np.float32)
    return out.reshape(B, S, D_MODEL).astype(np.float32)


# revision 7
# speedup vs baseline: 1.8908x; 1.3275x over previous
"""LoRA attention Bass kernel for 8x Trainium2 NeuronCores.

Sharding (Megatron tensor-parallel over heads):
  - Each of the 8 cores owns 2 heads (128 projection columns).
  - q/k/v projections column-sharded; out projection row-sharded;
    per-core partial outputs are summed on the host.
  - LoRA is merged into the base weights on the host (w_eff = w + a@u*scaling),
    which is exact up to fp32 rounding.

All matmul operands are bf16 (fp32/f32r matmuls trip the PE power throttle
to a 50% duty cycle; bf16 streams at 1 row/cycle at 2.4 GHz). PSUM
accumulation stays fp32.

Device schedule (per core):
  Phase 1 (proj): qT/kT computed transposed ([proj_col, seq]) from xT tiles;
      v computed in natural layout ([seq, proj_col]). Biases folded in via
      DVE tensor_scalar on the PSUM->SBUF copy. Projections for batch b+1 are
      interleaved into batch b's attention t-loop to fill PE idle slots
      (the t-loop is ACT-bound).
  Phase 2 (attention): S^T = K @ Q^T per (batch, head); scores for two
      t-chunks share one 2-bank PSUM tile so a single ACT exp instruction
      covers 1024 elements (halves ACT per-instruction overhead). P@V uses
      lhsT=[v | ones] so the softmax denominator falls out of the same
      matmul (row 64 of the PSUM output).
  Phase 3 (out-proj): recip via DVE reciprocal_approx_fast, broadcast to 64
      partitions via a K=1 ones matmul, normalize+cast to bf16 on DVE, then
      out = attnout @ Wo_slice accumulated in PSUM and DMA'd out.

PSUM budget (8 banks): s2 tag 2x[128,1024] (scores / bcast / out-proj) = 4,
  pv_a + pv_b = 2, proj accumulators (q/k/v rotating, bufs=2) = 2.
"""

import numpy as np
import ml_dtypes

import concourse.bass as bass
import concourse.mybir as mybir
import concourse.tile as tile
from concourse import bacc
from concourse.bass_utils import run_bass_kernel_spmd

F32 = mybir.dt.float32
F32R = mybir.dt.float32r
BF16 = mybir.dt.bfloat16
AF = mybir.ActivationFunctionType
ALU = mybir.AluOpType

N_CORES = 8

# Full-problem dims (hardcoded per spec)
D_MODEL = 1024
N_HEADS = 16
D_K = 64
LORA_R = 8
SCALING = 2.0
B = 4
S = 2048

NP_BF16 = ml_dtypes.bfloat16


class Cfg:
    def __init__(self, b=B, s=S, d=D_MODEL, cpc=128, dk=D_K):
        self.b = b                      # batches
        self.s = s                      # seq per batch
        self.d = d                      # model dim
        self.cpc = cpc                  # projection cols per core (2 heads x 64)
        self.dk = dk                    # head dim
        self.seq = b * s                # total rows
        self.nkc = d // 128             # k chunks for projections
        self.sc = 512                   # s-chunk width (free dim of matmuls)
        self.nsc = self.seq // self.sc  # s chunks over the whole input
        self.ntb = s // 128             # t chunks per batch
        self.ntp = self.ntb // 2        # t-chunk pairs per batch
        self.nsb = s // self.sc         # s chunks per batch
        self.ntc_g = self.seq // 128    # global t chunks


def _build_nc(cfg: Cfg):
    c = cfg
    dk = c.dk
    nc = bacc.Bacc("TRN2", target_bir_lowering=False, debug=False,
                   num_devices=N_CORES)

    xT = nc.dram_tensor("xT", [c.d, c.seq], BF16, kind="ExternalInput").ap()
    wq = nc.dram_tensor("wq", [c.d, c.cpc], BF16, kind="ExternalInput").ap()
    wk = nc.dram_tensor("wk", [c.d, c.cpc], BF16, kind="ExternalInput").ap()
    wv = nc.dram_tensor("wv", [c.d, c.cpc], BF16, kind="ExternalInput").ap()
    wo = nc.dram_tensor("wo", [c.cpc, c.d], BF16, kind="ExternalInput").ap()
    bq = nc.dram_tensor("bq", [c.cpc, 1], F32, kind="ExternalInput").ap()
    bk = nc.dram_tensor("bk", [c.cpc, 1], F32, kind="ExternalInput").ap()
    out = nc.dram_tensor("out", [c.seq, c.d], F32, kind="ExternalOutput").ap()

    xT_r = xT.rearrange("(kc p) s -> p kc s", p=128)

    with tile.TileContext(nc) as tc:
        with tc.tile_pool(name="persist", bufs=1) as persist:
            qT_sb = persist.tile([128, c.seq], BF16, tag="qT")
            kT_sb = persist.tile([128, c.seq], BF16, tag="kT")
            # v natural + ones cols: [0:64]=headA, 64=ones, 65:129=headB, 129=ones
            v_sb = persist.tile([128, c.ntc_g, 2 * dk + 2], BF16, tag="v")
            wq_sb = persist.tile([128, c.nkc, c.cpc], BF16, tag="wq")
            wk_sb = persist.tile([128, c.nkc, c.cpc], BF16, tag="wk")
            wv_sb = persist.tile([128, c.nkc, c.cpc], BF16, tag="wv")
            woA_sb = persist.tile([dk, c.d], BF16, tag="woA")
            woB_sb = persist.tile([dk, c.d], BF16, tag="woB")
            bq_sb = persist.tile([c.cpc, 1], F32, tag="bq")
            bk_sb = persist.tile([c.cpc, 1], F32, tag="bk")
            ones64 = persist.tile([1, dk], BF16, tag="ones64")

            nc.sync.dma_start(out=wq_sb[:], in_=wq.rearrange("(kc p) m -> p kc m", p=128))
            nc.sync.dma_start(out=wk_sb[:], in_=wk.rearrange("(kc p) m -> p kc m", p=128))
            nc.sync.dma_start(out=wv_sb[:], in_=wv.rearrange("(kc p) m -> p kc m", p=128))
            nc.sync.dma_start(out=woA_sb[:], in_=wo[0:dk, :])
            nc.sync.dma_start(out=woB_sb[:], in_=wo[dk:2 * dk, :])
            nc.sync.dma_start(out=bq_sb[:], in_=bq[:])
            nc.sync.dma_start(out=bk_sb[:], in_=bk[:])

            ones_f32 = persist.tile([128, 1], F32, tag="ones_f32")
            nc.vector.memset(ones_f32[:], 1.0)
            nc.vector.tensor_copy(ones64[:], ones_f32[0:1, :].to_broadcast([1, dk]))
            nc.vector.tensor_copy(
                v_sb[:, :, dk:dk + 1],
                ones_f32[:].unsqueeze(1).to_broadcast([128, c.ntc_g, 1]))
            nc.vector.tensor_copy(
                v_sb[:, :, 2 * dk + 1:2 * dk + 2],
                ones_f32[:].unsqueeze(1).to_broadcast([128, c.ntc_g, 1]))

            with tc.tile_pool(name="xin", bufs=2) as xpool, \
                 tc.tile_pool(name="ps2", bufs=2, space="PSUM") as ps2, \
                 tc.tile_pool(name="pvp", bufs=1, space="PSUM") as pvp, \
                 tc.tile_pool(name="prj", bufs=2, space="PSUM") as prj, \
                 tc.tile_pool(name="exp", bufs=3) as epool, \
                 tc.tile_pool(name="norm", bufs=2) as npool, \
                 tc.tile_pool(name="rec", bufs=2) as rpool, \
                 tc.tile_pool(name="osb", bufs=3) as opool:

                xc_tiles = {}

                def dma_x(sc_i):
                    x_t = xpool.tile([128, c.nkc, c.sc], BF16, tag="x",
                                     name=f"xc_{sc_i}")
                    s0 = sc_i * c.sc
                    nc.sync.dma_start(out=x_t[:], in_=xT_r[:, :, s0:s0 + c.sc])
                    xc_tiles[sc_i] = x_t

                # Per-chunk projection state (psum tiles held across stages)
                pstate = {}

                def proj_stage(sc_i, stage):
                    """Emit 1/8th of projection chunk sc_i (stages 0..7)."""
                    xc = xc_tiles[sc_i]
                    s0 = sc_i * c.sc
                    st = pstate.setdefault(sc_i, {})
                    if stage == 0:
                        q_ps = prj.tile([128, c.sc], F32, tag="prj",
                                        name=f"q_ps_{sc_i}")
                        st["q"] = q_ps
                        for kc in range(4):
                            nc.tensor.matmul(q_ps[:], wq_sb[:, kc, :],
                                             xc[:, kc, :],
                                             start=(kc == 0), stop=False)
                    elif stage == 1:
                        q_ps = st.pop("q")
                        for kc in range(4, c.nkc):
                            nc.tensor.matmul(q_ps[:], wq_sb[:, kc, :],
                                             xc[:, kc, :],
                                             start=False, stop=(kc == c.nkc - 1))
                        nc.vector.tensor_scalar(
                            qT_sb[:, s0:s0 + c.sc], q_ps[:], bq_sb[:], None,
                            ALU.add)
                    elif stage == 2:
                        k_ps = prj.tile([128, c.sc], F32, tag="prj",
                                        name=f"k_ps_{sc_i}")
                        st["k"] = k_ps
                        for kc in range(4):
                            nc.tensor.matmul(k_ps[:], wk_sb[:, kc, :],
                                             xc[:, kc, :],
                                             start=(kc == 0), stop=False)
                    elif stage == 3:
                        k_ps = st.pop("k")
                        for kc in range(4, c.nkc):
                            nc.tensor.matmul(k_ps[:], wk_sb[:, kc, :],
                                             xc[:, kc, :],
                                             start=False, stop=(kc == c.nkc - 1))
                        nc.vector.tensor_scalar(
                            kT_sb[:, s0:s0 + c.sc], k_ps[:], bk_sb[:], None,
                            ALU.add)
                    elif stage in (4, 5, 6):
                        if stage == 4:
                            v_ps = prj.tile([128, 4, 128], F32, tag="prj",
                                            name=f"v_ps_{sc_i}")
                            st["v"] = v_ps
                        v_ps = st["v"]
                        # One accumulation group for the whole bank: start
                        # zeroes the full 2KB zero region, so only the very
                        # first matmul may set start and only the last stop.
                        for kc in range(2 * (stage - 4), 2 * (stage - 4) + 2):
                            for j in range(4):
                                nc.tensor.matmul(
                                    v_ps[:, j, :],
                                    xc[:, kc, j * 128:(j + 1) * 128],
                                    wv_sb[:, kc, :],
                                    start=(kc == 0 and j == 0), stop=False,
                                    skip_group_check=True)
                    else:  # stage 7
                        v_ps = st.pop("v")
                        for kc in (6, 7):
                            for j in range(4):
                                nc.tensor.matmul(
                                    v_ps[:, j, :],
                                    xc[:, kc, j * 128:(j + 1) * 128],
                                    wv_sb[:, kc, :],
                                    start=False, stop=(kc == 7 and j == 3),
                                    skip_group_check=True)
                        tc0 = sc_i * 4
                        for j in range(4):
                            nc.vector.tensor_copy(
                                v_sb[:, tc0 + j, 0:dk], v_ps[:, j, 0:dk])
                            nc.vector.tensor_copy(
                                v_sb[:, tc0 + j, dk + 1:2 * dk + 1],
                                v_ps[:, j, dk:2 * dk])
                        del xc_tiles[sc_i]
                        pstate.pop(sc_i, None)

                # ---------------- batch 0 projections upfront ----------------
                for sc_i in range(c.nsb):
                    dma_x(sc_i)
                for sc_i in range(c.nsb):
                    for stage in range(8):
                        proj_stage(sc_i, stage)
                # prefetch x for the first interleaved proj chunk
                if c.b > 1:
                    dma_x(c.nsb)

                # ---------------- main loop ----------------
                for b_i in range(c.b):
                    for sb in range(c.nsb):
                        s0 = b_i * c.s + sb * c.sc
                        proj_c = (b_i + 1) * c.nsb + sb if b_i + 1 < c.b else None
                        pva = pvp.tile([dk + 1, c.sc], F32, tag="pva")
                        pvb = pvp.tile([dk + 1, c.sc], F32, tag="pvb")
                        for tp in range(c.ntp):
                            t0 = b_i * c.s + (2 * tp) * 128
                            t1 = t0 + 128
                            s2a = ps2.tile([128, 2 * c.sc], F32, tag="s2",
                                           name=f"s2a_{b_i}_{sb}_{tp}")
                            nc.tensor.matmul(
                                s2a[:, 0:c.sc], kT_sb[0:dk, t0:t0 + 128],
                                qT_sb[0:dk, s0:s0 + c.sc],
                                start=True, stop=True)
                            nc.tensor.matmul(
                                s2a[:, c.sc:2 * c.sc], kT_sb[0:dk, t1:t1 + 128],
                                qT_sb[0:dk, s0:s0 + c.sc],
                                start=True, stop=True)
                            e2a = epool.tile([128, 2 * c.sc], BF16, tag="e2",
                                             name=f"e2a_{b_i}_{sb}_{tp}")
                            nc.scalar.activation(e2a[:], s2a[:], AF.Exp,
                                                 scale=1.0 / np.sqrt(dk))
                            s2b = ps2.tile([128, 2 * c.sc], F32, tag="s2",
                                           name=f"s2b_{b_i}_{sb}_{tp}")
                            nc.tensor.matmul(
                                s2b[:, 0:c.sc], kT_sb[dk:2 * dk, t0:t0 + 128],
                                qT_sb[dk:2 * dk, s0:s0 + c.sc],
                                start=True, stop=True)
                            nc.tensor.matmul(
                                s2b[:, c.sc:2 * c.sc], kT_sb[dk:2 * dk, t1:t1 + 128],
                                qT_sb[dk:2 * dk, s0:s0 + c.sc],
                                start=True, stop=True)
                            e2b = epool.tile([128, 2 * c.sc], BF16, tag="e2",
                                             name=f"e2b_{b_i}_{sb}_{tp}")
                            nc.scalar.activation(e2b[:], s2b[:], AF.Exp,
                                                 scale=1.0 / np.sqrt(dk))
                            tca = b_i * c.ntb + 2 * tp
                            tcb = tca + 1
                            nc.tensor.matmul(
                                pva[:], v_sb[:, tca, 0:dk + 1], e2a[:, 0:c.sc],
                                start=(tp == 0), stop=False)
                            nc.tensor.matmul(
                                pva[:], v_sb[:, tcb, 0:dk + 1],
                                e2a[:, c.sc:2 * c.sc],
                                start=False, stop=(tp == c.ntp - 1))
                            nc.tensor.matmul(
                                pvb[:], v_sb[:, tca, dk + 1:2 * dk + 2],
                                e2b[:, 0:c.sc],
                                start=(tp == 0), stop=False)
                            nc.tensor.matmul(
                                pvb[:], v_sb[:, tcb, dk + 1:2 * dk + 2],
                                e2b[:, c.sc:2 * c.sc],
                                start=False, stop=(tp == c.ntp - 1))
                            if proj_c is not None:
                                for st_i in range(tp * 8 // c.ntp,
                                                  (tp + 1) * 8 // c.ntp):
                                    proj_stage(proj_c, st_i)

                        # ---- normalize ----
                        den_a = rpool.tile([1, c.sc], F32, tag="den")
                        den_b = rpool.tile([1, c.sc], F32, tag="den")
                        nc.vector.tensor_copy(den_a[:], pva[dk:dk + 1, :])
                        nc.vector.tensor_copy(den_b[:], pvb[dk:dk + 1, :])
                        rec_af = rpool.tile([1, c.sc], F32, tag="recf")
                        rec_bf = rpool.tile([1, c.sc], F32, tag="recf")
                        nc.vector.reciprocal_approx_fast(
                            out=rec_af[:], in_=den_a[:])
                        nc.vector.reciprocal_approx_fast(
                            out=rec_bf[:], in_=den_b[:])
                        rec_ab = rpool.tile([1, c.sc], BF16, tag="recb")
                        rec_bb = rpool.tile([1, c.sc], BF16, tag="recb")
                        nc.vector.tensor_copy(rec_ab[:], rec_af[:])
                        nc.vector.tensor_copy(rec_bb[:], rec_bf[:])
                        bc2 = ps2.tile([128, 2 * c.sc], F32, tag="s2",
                                       name=f"bc2_{b_i}_{sb}")
                        nc.tensor.matmul(bc2[0:dk, 0:c.sc], ones64[:],
                                         rec_ab[:], start=True, stop=True)
                        nc.tensor.matmul(bc2[0:dk, c.sc:2 * c.sc], ones64[:],
                                         rec_bb[:], start=True, stop=True)
                        bcs = npool.tile([dk, 2 * c.sc], F32, tag="bcs")
                        nc.vector.tensor_copy(bcs[:], bc2[0:dk, :])
                        norm_a = npool.tile([dk, c.sc], BF16, tag="na")
                        norm_b = npool.tile([dk, c.sc], BF16, tag="nb")
                        nc.vector.tensor_tensor(
                            norm_a[:], pva[0:dk, :], bcs[:, 0:c.sc],
                            ALU.mult)
                        nc.vector.tensor_tensor(
                            norm_b[:], pvb[0:dk, :], bcs[:, c.sc:2 * c.sc],
                            ALU.mult)

                        # ---- out projection ----
                        for j in range(c.sc // 128):
                            o2 = ps2.tile([128, 2 * c.sc], F32, tag="s2",
                                          name=f"o2_{b_i}_{sb}_{j}")
                            for e in range(2):
                                nc.tensor.matmul(
                                    o2[:, e * c.sc:(e + 1) * c.sc],
                                    norm_a[:, j * 128:(j + 1) * 128],
                                    woA_sb[:, e * c.sc:(e + 1) * c.sc],
                                    start=True, stop=False)
                                nc.tensor.matmul(
                                    o2[:, e * c.sc:(e + 1) * c.sc],
                                    norm_b[:, j * 128:(j + 1) * 128],
                                    woB_sb[:, e * c.sc:(e + 1) * c.sc],
                                    start=False, stop=True)
                            osb = opool.tile([128, c.d], F32, tag="osb",
                                             name=f"osb_{b_i}_{sb}_{j}")
                            nc.vector.tensor_copy(osb[:], o2[:])
                            nc.sync.dma_start(
                                out=out[s0 + j * 128:s0 + (j + 1) * 128, :],
                                in_=osb[:])

                        # prefetch x for the next interleaved proj chunk
                        if proj_c is not None:
                            nxt = proj_c + 1
                            if nxt < c.nsc and nxt not in xc_tiles:
                                dma_x(nxt)

    nc.compile()
    return nc


_NC_CACHE = {}


def get_nc(cfg: Cfg | None = None):
    cfg = cfg or Cfg()
    key = (cfg.b, cfg.s, cfg.d, cfg.cpc, cfg.dk)
    if key not in _NC_CACHE:
        _NC_CACHE[key] = _build_nc(cfg)
    return _NC_CACHE[key]


def make_in_maps(x, w_q, b_q, w_k, b_k, w_v, b_v, w_o, b_o,
                 a_q, u_q, a_k, u_k, a_v, u_v, cfg: Cfg | None = None):
    """Host-side prep: merge LoRA, transpose x, cast to bf16, shard."""
    c = cfg or Cfg()
    x = np.asarray(x, np.float32)
    w_o = np.asarray(w_o, np.float32)

    def merge(w, a, u):
        return (np.asarray(w, np.float64)
                + (np.asarray(a, np.float64) @ np.asarray(u, np.float64))
                * SCALING).astype(np.float32)

    wq_eff = merge(w_q, a_q, u_q)
    wk_eff = merge(w_k, a_k, u_k)
    wv_eff = merge(w_v, a_v, u_v)

    xT = np.ascontiguousarray(x.reshape(c.seq, c.d).T).astype(NP_BF16)
    b_q = np.asarray(b_q, np.float32)
    b_k = np.asarray(b_k, np.float32)
    in_maps = []
    for i in range(N_CORES):
        sl = slice(i * c.cpc, (i + 1) * c.cpc)
        in_maps.append({
            "xT": xT,
            "wq": np.ascontiguousarray(wq_eff[:, sl]).astype(NP_BF16),
            "wk": np.ascontiguousarray(wk_eff[:, sl]).astype(NP_BF16),
            "wv": np.ascontiguousarray(wv_eff[:, sl]).astype(NP_BF16),
            "wo": np.ascontiguousarray(w_o[sl, :]).astype(NP_BF16),
            "bq": np.ascontiguousarray(b_q[sl]).reshape(c.cpc, 1),
            "bk": np.ascontiguousarray(b_k[sl]).reshape(c.cpc, 1),
        })
    return in_maps


def kernel(x, w_q, b_q, w_k, b_k, w_v, b_v, w_o, b_o,
           a_q, u_q, a_k, u_k, a_v, u_v):
    cfg = Cfg()
    c = cfg
    in_maps = make_in_maps(x, w_q, b_q, w_k, b_k, w_v, b_v, w_o, b_o,
                           a_q, u_q, a_k, u_k, a_v, u_v, cfg)
    nc = get_nc(cfg)
    res = run_bass_kernel_spmd(nc, in_maps, list(range(N_CORES)))
    out = np.zeros((c.seq, c.d), np.float32)
    for i in range(N_CORES):
        out += res.results[i]["out"]
    # v-bias rides through softmax as a constant row; b_o is plain bias
    b_v = np.asarray(b_v, np.float32)
    b_o = np.asarray(b_o, np.float32)
    w_o = np.asarray(w_o, np.float32)
    out += (b_v @ w_o + b_o).astype(np.float32)
    return out.reshape(B, S, D_MODEL).astype(np.float32)


# revision 13
# speedup vs baseline: 1.9054x; 1.0077x over previous
"""LoRA attention Bass kernel for 8x Trainium2 NeuronCores.

Sharding (Megatron tensor-parallel over heads):
  - Each of the 8 cores owns 2 heads (128 projection columns).
  - q/k/v projections column-sharded; out projection row-sharded;
    per-core partial outputs are summed on the host.
  - LoRA is merged into the base weights on the host (w_eff = w + a@u*scaling),
    which is exact up to fp32 rounding.

All matmul operands are bf16 (fp32/f32r matmuls trip the PE power throttle
to a 50% duty cycle; bf16 streams at 1 row/cycle at 2.4 GHz). PSUM
accumulation stays fp32.

Device schedule (per core):
  Phase 1 (proj): qT/kT computed transposed ([proj_col, seq]) from xT tiles;
      v computed in natural layout ([seq, proj_col]). Biases folded in via
      DVE tensor_scalar on the PSUM->SBUF copy. Projections for batch b+1 are
      interleaved into batch b's attention t-loop to fill PE idle slots
      (the t-loop is ACT-bound).
  Phase 2 (attention): S^T = K @ Q^T per (batch, head); scores for two
      t-chunks share one 2-bank PSUM tile so a single ACT exp instruction
      covers 1024 elements (halves ACT per-instruction overhead). P@V uses
      lhsT=[v | ones] so the softmax denominator falls out of the same
      matmul (row 64 of the PSUM output).
  Phase 3 (out-proj): recip via DVE reciprocal_approx_fast, broadcast to 64
      partitions via a K=1 ones matmul, normalize+cast to bf16 on DVE, then
      out = attnout @ Wo_slice accumulated in PSUM and DMA'd out.

PSUM budget (8 banks): s2 tag 2x[128,1024] (scores / bcast / out-proj) = 4,
  pv_a + pv_b = 2, proj accumulators (q/k/v rotating, bufs=2) = 2.
"""

import numpy as np
import ml_dtypes

import concourse.bass as bass
import concourse.mybir as mybir
import concourse.tile as tile
from concourse import bacc
from concourse.bass_utils import run_bass_kernel_spmd

F32 = mybir.dt.float32
F32R = mybir.dt.float32r
BF16 = mybir.dt.bfloat16
AF = mybir.ActivationFunctionType
ALU = mybir.AluOpType

N_CORES = 8

# Full-problem dims (hardcoded per spec)
D_MODEL = 1024
N_HEADS = 16
D_K = 64
LORA_R = 8
SCALING = 2.0
B = 4
S = 2048

NP_BF16 = ml_dtypes.bfloat16


class Cfg:
    def __init__(self, b=B, s=S, d=D_MODEL, cpc=128, dk=D_K):
        self.b = b                      # batches
        self.s = s                      # seq per batch
        self.d = d                      # model dim
        self.cpc = cpc                  # projection cols per core (2 heads x 64)
        self.dk = dk                    # head dim
        self.seq = b * s                # total rows
        self.nkc = d // 128             # k chunks for projections
        self.sc = 512                   # s-chunk width (free dim of matmuls)
        self.nsc = self.seq // self.sc  # s chunks over the whole input
        self.ntb = s // 128             # t chunks per batch
        self.ntp = self.ntb // 2        # t-chunk pairs per batch
        self.nsb = s // self.sc         # s chunks per batch
        self.ntc_g = self.seq // 128    # global t chunks


def _build_nc(cfg: Cfg):
    c = cfg
    dk = c.dk
    nc = bacc.Bacc("TRN2", target_bir_lowering=False, debug=False,
                   num_devices=N_CORES)

    xT = nc.dram_tensor("xT", [c.d, c.seq], BF16, kind="ExternalInput").ap()
    wq = nc.dram_tensor("wq", [c.d, c.cpc], BF16, kind="ExternalInput").ap()
    wk = nc.dram_tensor("wk", [c.d, c.cpc], BF16, kind="ExternalInput").ap()
    wv = nc.dram_tensor("wv", [c.d, c.cpc], BF16, kind="ExternalInput").ap()
    wo = nc.dram_tensor("wo", [c.cpc, c.d], BF16, kind="ExternalInput").ap()
    bq = nc.dram_tensor("bq", [c.cpc, 1], F32, kind="ExternalInput").ap()
    bk = nc.dram_tensor("bk", [c.cpc, 1], F32, kind="ExternalInput").ap()
    out = nc.dram_tensor("out", [c.seq, c.d], F32, kind="ExternalOutput").ap()

    xT_r = xT.rearrange("(kc p) s -> p kc s", p=128)

    with tile.TileContext(nc) as tc:
        with tc.tile_pool(name="persist", bufs=1) as persist:
            qT_sb = persist.tile([128, c.seq], BF16, tag="qT")
            kT_sb = persist.tile([128, c.seq], BF16, tag="kT")
            # v natural + ones cols: [0:64]=headA, 64=ones, 65:129=headB, 129=ones
            v_sb = persist.tile([128, c.ntc_g, 2 * dk + 2], BF16, tag="v")
            wq_sb = persist.tile([128, c.nkc, c.cpc], BF16, tag="wq")
            wk_sb = persist.tile([128, c.nkc, c.cpc], BF16, tag="wk")
            wv_sb = persist.tile([128, c.nkc, c.cpc], BF16, tag="wv")
            woA_sb = persist.tile([dk, c.d], BF16, tag="woA")
            woB_sb = persist.tile([dk, c.d], BF16, tag="woB")
            bq_sb = persist.tile([c.cpc, 1], F32, tag="bq")
            bk_sb = persist.tile([c.cpc, 1], F32, tag="bk")
            ones64 = persist.tile([1, dk], BF16, tag="ones64")

            nc.sync.dma_start(out=wq_sb[:], in_=wq.rearrange("(kc p) m -> p kc m", p=128))
            nc.sync.dma_start(out=wk_sb[:], in_=wk.rearrange("(kc p) m -> p kc m", p=128))
            nc.sync.dma_start(out=wv_sb[:], in_=wv.rearrange("(kc p) m -> p kc m", p=128))
            nc.sync.dma_start(out=woA_sb[:], in_=wo[0:dk, :])
            nc.sync.dma_start(out=woB_sb[:], in_=wo[dk:2 * dk, :])
            nc.sync.dma_start(out=bq_sb[:], in_=bq[:])
            nc.sync.dma_start(out=bk_sb[:], in_=bk[:])

            ones_f32 = persist.tile([128, 1], F32, tag="ones_f32")
            nc.vector.memset(ones_f32[:], 1.0)
            nc.vector.tensor_copy(ones64[:], ones_f32[0:1, :].to_broadcast([1, dk]))
            nc.vector.tensor_copy(
                v_sb[:, :, dk:dk + 1],
                ones_f32[:].unsqueeze(1).to_broadcast([128, c.ntc_g, 1]))
            nc.vector.tensor_copy(
                v_sb[:, :, 2 * dk + 1:2 * dk + 2],
                ones_f32[:].unsqueeze(1).to_broadcast([128, c.ntc_g, 1]))

            with tc.tile_pool(name="xin", bufs=2) as xpool, \
                 tc.tile_pool(name="ps2", bufs=2, space="PSUM") as ps2, \
                 tc.tile_pool(name="pvp", bufs=1, space="PSUM") as pvp, \
                 tc.tile_pool(name="prj", bufs=2, space="PSUM") as prj, \
                 tc.tile_pool(name="exp", bufs=3) as epool, \
                 tc.tile_pool(name="norm", bufs=2) as npool, \
                 tc.tile_pool(name="rec", bufs=2) as rpool, \
                 tc.tile_pool(name="osb", bufs=3) as opool:

                xc_tiles = {}

                def dma_x(sc_i):
                    x_t = xpool.tile([128, c.nkc, c.sc], BF16, tag="x",
                                     name=f"xc_{sc_i}")
                    s0 = sc_i * c.sc
                    nc.sync.dma_start(out=x_t[:], in_=xT_r[:, :, s0:s0 + c.sc])
                    xc_tiles[sc_i] = x_t

                # Per-chunk projection state (psum tiles held across stages)
                pstate = {}

                def proj_stage(sc_i, stage):
                    """Emit 1/8th of projection chunk sc_i (stages 0..7)."""
                    xc = xc_tiles[sc_i]
                    s0 = sc_i * c.sc
                    st = pstate.setdefault(sc_i, {})
                    if stage == 0:
                        q_ps = prj.tile([128, c.sc], F32, tag="prj",
                                        name=f"q_ps_{sc_i}")
                        st["q"] = q_ps
                        for kc in range(4):
                            nc.tensor.matmul(q_ps[:], wq_sb[:, kc, :],
                                             xc[:, kc, :],
                                             start=(kc == 0), stop=False)
                    elif stage == 1:
                        q_ps = st.pop("q")
                        for kc in range(4, c.nkc):
                            nc.tensor.matmul(q_ps[:], wq_sb[:, kc, :],
                                             xc[:, kc, :],
                                             start=False, stop=(kc == c.nkc - 1))
                        nc.vector.tensor_scalar(
                            qT_sb[:, s0:s0 + c.sc], q_ps[:], bq_sb[:], None,
                            ALU.add)
                    elif stage == 2:
                        k_ps = prj.tile([128, c.sc], F32, tag="prj",
                                        name=f"k_ps_{sc_i}")
                        st["k"] = k_ps
                        for kc in range(4):
                            nc.tensor.matmul(k_ps[:], wk_sb[:, kc, :],
                                             xc[:, kc, :],
                                             start=(kc == 0), stop=False)
                    elif stage == 3:
                        k_ps = st.pop("k")
                        for kc in range(4, c.nkc):
                            nc.tensor.matmul(k_ps[:], wk_sb[:, kc, :],
                                             xc[:, kc, :],
                                             start=False, stop=(kc == c.nkc - 1))
                        nc.vector.tensor_scalar(
                            kT_sb[:, s0:s0 + c.sc], k_ps[:], bk_sb[:], None,
                            ALU.add)
                    elif stage in (4, 5, 6):
                        if stage == 4:
                            v_ps = prj.tile([128, 4, 128], F32, tag="prj",
                                            name=f"v_ps_{sc_i}")
                            st["v"] = v_ps
                        v_ps = st["v"]
                        # One accumulation group for the whole bank: start
                        # zeroes the full 2KB zero region, so only the very
                        # first matmul may set start and only the last stop.
                        for kc in range(2 * (stage - 4), 2 * (stage - 4) + 2):
                            for j in range(4):
                                nc.tensor.matmul(
                                    v_ps[:, j, :],
                                    xc[:, kc, j * 128:(j + 1) * 128],
                                    wv_sb[:, kc, :],
                                    start=(kc == 0 and j == 0), stop=False,
                                    skip_group_check=True)
                    else:  # stage 7
                        v_ps = st.pop("v")
                        for kc in (6, 7):
                            for j in range(4):
                                nc.tensor.matmul(
                                    v_ps[:, j, :],
                                    xc[:, kc, j * 128:(j + 1) * 128],
                                    wv_sb[:, kc, :],
                                    start=False, stop=(kc == 7 and j == 3),
                                    skip_group_check=True)
                        tc0 = sc_i * 4
                        for j in range(4):
                            nc.vector.tensor_copy(
                                v_sb[:, tc0 + j, 0:dk], v_ps[:, j, 0:dk])
                            nc.vector.tensor_copy(
                                v_sb[:, tc0 + j, dk + 1:2 * dk + 1],
                                v_ps[:, j, dk:2 * dk])
                        del xc_tiles[sc_i]
                        pstate.pop(sc_i, None)

                # ---------------- batch 0 projections upfront ----------------
                for sc_i in range(c.nsb):
                    dma_x(sc_i)
                for sc_i in range(c.nsb):
                    for stage in range(8):
                        proj_stage(sc_i, stage)
                # prefetch x for the first interleaved proj chunk
                if c.b > 1:
                    dma_x(c.nsb)

                # ---------------- main loop ----------------
                for b_i in range(c.b):
                    for sb in range(c.nsb):
                        s0 = b_i * c.s + sb * c.sc
                        proj_c = (b_i + 1) * c.nsb + sb if b_i + 1 < c.b else None
                        pva = pvp.tile([dk + 1, c.sc], F32, tag="pva")
                        pvb = pvp.tile([dk + 1, c.sc], F32, tag="pvb")
                        for tp in range(c.ntp):
                            t0 = b_i * c.s + (2 * tp) * 128
                            t1 = t0 + 128
                            s2a = ps2.tile([128, 2 * c.sc], F32, tag="s2",
                                           name=f"s2a_{b_i}_{sb}_{tp}")
                            nc.tensor.matmul(
                                s2a[:, 0:c.sc], kT_sb[0:dk, t0:t0 + 128],
                                qT_sb[0:dk, s0:s0 + c.sc],
                                start=True, stop=True)
                            nc.tensor.matmul(
                                s2a[:, c.sc:2 * c.sc], kT_sb[0:dk, t1:t1 + 128],
                                qT_sb[0:dk, s0:s0 + c.sc],
                                start=True, stop=True)
                            e2a = epool.tile([128, 2 * c.sc], BF16, tag="e2",
                                             name=f"e2a_{b_i}_{sb}_{tp}")
                            nc.scalar.activation(e2a[:], s2a[:], AF.Exp,
                                                 scale=1.0 / np.sqrt(dk))
                            s2b = ps2.tile([128, 2 * c.sc], F32, tag="s2",
                                           name=f"s2b_{b_i}_{sb}_{tp}")
                            nc.tensor.matmul(
                                s2b[:, 0:c.sc], kT_sb[dk:2 * dk, t0:t0 + 128],
                                qT_sb[dk:2 * dk, s0:s0 + c.sc],
                                start=True, stop=True)
                            nc.tensor.matmul(
                                s2b[:, c.sc:2 * c.sc], kT_sb[dk:2 * dk, t1:t1 + 128],
                                qT_sb[dk:2 * dk, s0:s0 + c.sc],
                                start=True, stop=True)
                            e2b = epool.tile([128, 2 * c.sc], BF16, tag="e2",
                                             name=f"e2b_{b_i}_{sb}_{tp}")
                            nc.scalar.activation(e2b[:], s2b[:], AF.Exp,
                                                 scale=1.0 / np.sqrt(dk))
                            tca = b_i * c.ntb + 2 * tp
                            tcb = tca + 1
                            nc.tensor.matmul(
                                pva[:], v_sb[:, tca, 0:dk + 1], e2a[:, 0:c.sc],
                                start=(tp == 0), stop=False)
                            nc.tensor.matmul(
                                pva[:], v_sb[:, tcb, 0:dk + 1],
                                e2a[:, c.sc:2 * c.sc],
                                start=False, stop=(tp == c.ntp - 1))
                            nc.tensor.matmul(
                                pvb[:], v_sb[:, tca, dk + 1:2 * dk + 2],
                                e2b[:, 0:c.sc],
                                start=(tp == 0), stop=False)
                            nc.tensor.matmul(
                                pvb[:], v_sb[:, tcb, dk + 1:2 * dk + 2],
                                e2b[:, c.sc:2 * c.sc],
                                start=False, stop=(tp == c.ntp - 1))
                            if proj_c is not None:
                                for st_i in range(tp * 8 // c.ntp,
                                                  (tp + 1) * 8 // c.ntp):
                                    proj_stage(proj_c, st_i)

                        # ---- normalize ----
                        # NB: reciprocal_approx_fast directly on the PSUM rows
                        # (pva[64:65]) returns garbage on HW even though an
                        # isolated probe of the same AP works — stage the
                        # denominators through SBUF first.
                        den_a = rpool.tile([1, c.sc], F32, tag="den")
                        den_b = rpool.tile([1, c.sc], F32, tag="den")
                        nc.vector.tensor_copy(den_a[:], pva[dk:dk + 1, :])
                        nc.vector.tensor_copy(den_b[:], pvb[dk:dk + 1, :])
                        rec_af = rpool.tile([1, c.sc], F32, tag="recf")
                        rec_bf = rpool.tile([1, c.sc], F32, tag="recf")
                        nc.vector.reciprocal_approx_fast(
                            out=rec_af[:], in_=den_a[:])
                        nc.vector.reciprocal_approx_fast(
                            out=rec_bf[:], in_=den_b[:])
                        rec_ab = rpool.tile([1, c.sc], BF16, tag="recb")
                        rec_bb = rpool.tile([1, c.sc], BF16, tag="recb")
                        nc.vector.tensor_copy(rec_ab[:], rec_af[:])
                        nc.vector.tensor_copy(rec_bb[:], rec_bf[:])
                        bc2 = ps2.tile([128, 2 * c.sc], F32, tag="s2",
                                       name=f"bc2_{b_i}_{sb}")
                        nc.tensor.matmul(bc2[0:dk, 0:c.sc], ones64[:],
                                         rec_ab[:], start=True, stop=True)
                        nc.tensor.matmul(bc2[0:dk, c.sc:2 * c.sc], ones64[:],
                                         rec_bb[:], start=True, stop=True)
                        bcs = npool.tile([dk, 2 * c.sc], F32, tag="bcs")
                        nc.vector.tensor_copy(bcs[:], bc2[0:dk, :])
                        norm_a = npool.tile([dk, c.sc], BF16, tag="na")
                        norm_b = npool.tile([dk, c.sc], BF16, tag="nb")
                        nc.vector.tensor_tensor(
                            norm_a[:], pva[0:dk, :], bcs[:, 0:c.sc],
                            ALU.mult)
                        nc.vector.tensor_tensor(
                            norm_b[:], pvb[0:dk, :], bcs[:, c.sc:2 * c.sc],
                            ALU.mult)

                        # ---- out projection ----
                        for j in range(c.sc // 128):
                            o2 = ps2.tile([128, 2 * c.sc], F32, tag="s2",
                                          name=f"o2_{b_i}_{sb}_{j}")
                            for e in range(2):
                                nc.tensor.matmul(
                                    o2[:, e * c.sc:(e + 1) * c.sc],
                                    norm_a[:, j * 128:(j + 1) * 128],
                                    woA_sb[:, e * c.sc:(e + 1) * c.sc],
                                    start=True, stop=False)
                                nc.tensor.matmul(
                                    o2[:, e * c.sc:(e + 1) * c.sc],
                                    norm_b[:, j * 128:(j + 1) * 128],
                                    woB_sb[:, e * c.sc:(e + 1) * c.sc],
                                    start=False, stop=True)
                            osb = opool.tile([128, c.d], F32, tag="osb",
                                             name=f"osb_{b_i}_{sb}_{j}")
                            nc.scalar.copy(osb[:], o2[:])
                            nc.sync.dma_start(
                                out=out[s0 + j * 128:s0 + (j + 1) * 128, :],
                                in_=osb[:])

                        # prefetch x for the next interleaved proj chunk
                        if proj_c is not None:
                            nxt = proj_c + 1
                            if nxt < c.nsc and nxt not in xc_tiles:
                                dma_x(nxt)

    nc.compile()
    return nc


_NC_CACHE = {}


def get_nc(cfg: Cfg | None = None):
    cfg = cfg or Cfg()
    key = (cfg.b, cfg.s, cfg.d, cfg.cpc, cfg.dk)
    if key not in _NC_CACHE:
        _NC_CACHE[key] = _build_nc(cfg)
    return _NC_CACHE[key]


def make_in_maps(x, w_q, b_q, w_k, b_k, w_v, b_v, w_o, b_o,
                 a_q, u_q, a_k, u_k, a_v, u_v, cfg: Cfg | None = None):
    """Host-side prep: merge LoRA, transpose x, cast to bf16, shard."""
    c = cfg or Cfg()
    x = np.asarray(x, np.float32)
    w_o = np.asarray(w_o, np.float32)

    def merge(w, a, u):
        return (np.asarray(w, np.float64)
                + (np.asarray(a, np.float64) @ np.asarray(u, np.float64))
                * SCALING).astype(np.float32)

    wq_eff = merge(w_q, a_q, u_q)
    wk_eff = merge(w_k, a_k, u_k)
    wv_eff = merge(w_v, a_v, u_v)

    xT = np.ascontiguousarray(x.reshape(c.seq, c.d).T).astype(NP_BF16)
    b_q = np.asarray(b_q, np.float32)
    b_k = np.asarray(b_k, np.float32)
    in_maps = []
    for i in range(N_CORES):
        sl = slice(i * c.cpc, (i + 1) * c.cpc)
        in_maps.append({
            "xT": xT,
            "wq": np.ascontiguousarray(wq_eff[:, sl]).astype(NP_BF16),
            "wk": np.ascontiguousarray(wk_eff[:, sl]).astype(NP_BF16),
            "wv": np.ascontiguousarray(wv_eff[:, sl]).astype(NP_BF16),
            "wo": np.ascontiguousarray(w_o[sl, :]).astype(NP_BF16),
            "bq": np.ascontiguousarray(b_q[sl]).reshape(c.cpc, 1),
            "bk": np.ascontiguousarray(b_k[sl]).reshape(c.cpc, 1),
        })
    return in_maps


def kernel(x, w_q, b_q, w_k, b_k, w_v, b_v, w_o, b_o,
           a_q, u_q, a_k, u_k, a_v, u_v):
    cfg = Cfg()
    c = cfg
    in_maps = make_in_maps(x, w_q, b_q, w_k, b_k, w_v, b_v, w_o, b_o,
                           a_q, u_q, a_k, u_k, a_v, u_v, cfg)
    nc = get_nc(cfg)
    res = run_bass_kernel_spmd(nc, in_maps, list(range(N_CORES)))
    out = np.zeros((c.seq, c.d), np.float32)
    for i in range(N_CORES):
        out += res.results[i]["out"]
    # v-bias rides through softmax as a constant row; b_o is plain bias
    b_v = np.asarray(b_v, np.float32)
    b_o = np.asarray(b_o, np.float32)
    w_o = np.asarray(w_o, np.float32)
    out += (b_v @ w_o + b_o).astype(np.float32)
    return out.reshape(B, S, D_MODEL).astype(np.float32)


# revision 17
# speedup vs baseline: 2.1187x; 1.1119x over previous
"""LoRA attention Bass kernel for 8x Trainium2 NeuronCores.

Sharding (Megatron tensor-parallel over heads):
  - Each of the 8 cores owns 2 heads (128 projection columns).
  - q/k/v projections column-sharded; out projection row-sharded;
    per-core partial outputs are summed on the host.
  - LoRA is merged into the base weights on the host (w_eff = w + a@u*scaling),
    which is exact up to fp32 rounding.

All matmul operands are bf16 (fp32/f32r matmuls trip the PE power throttle
to a 50% duty cycle; bf16 streams at 1 row/cycle at 2.4 GHz). PSUM
accumulation stays fp32.

Device schedule (per core):
  Phase 1 (proj): qT/kT computed transposed ([proj_col, seq]) from xT tiles;
      v computed in natural layout ([seq, proj_col]). Biases folded in via
      DVE tensor_scalar on the PSUM->SBUF copy. Projections for batch b+1 are
      interleaved into batch b's attention t-loop to fill PE idle slots
      (the t-loop is ACT-bound).
  Phase 2 (attention): S^T = K @ Q^T per (batch, head); scores for two
      t-chunks share one 2-bank PSUM tile so a single ACT exp instruction
      covers 1024 elements (halves ACT per-instruction overhead). P@V uses
      lhsT=[v | ones] so the softmax denominator falls out of the same
      matmul (row 64 of the PSUM output).
  Phase 3 (out-proj): recip via DVE reciprocal_approx_fast, broadcast to 64
      partitions via a K=1 ones matmul, normalize+cast to bf16 on DVE, then
      out = attnout @ Wo_slice accumulated in PSUM and DMA'd out.

PSUM budget (8 banks): s2 tag 2x[128,1024] (scores / bcast / out-proj) = 4,
  pv_a + pv_b = 2, proj accumulators (q/k/v rotating, bufs=2) = 2.
"""

import numpy as np
import ml_dtypes

import concourse.bass as bass
import concourse.mybir as mybir
import concourse.tile as tile
from concourse import bacc
from concourse.bass_utils import run_bass_kernel_spmd

F32 = mybir.dt.float32
F32R = mybir.dt.float32r
BF16 = mybir.dt.bfloat16
AF = mybir.ActivationFunctionType
ALU = mybir.AluOpType

N_CORES = 8

# Full-problem dims (hardcoded per spec)
D_MODEL = 1024
N_HEADS = 16
D_K = 64
LORA_R = 8
SCALING = 2.0
B = 4
S = 2048

NP_BF16 = ml_dtypes.bfloat16


class Cfg:
    def __init__(self, b=B, s=S, d=D_MODEL, cpc=128, dk=D_K):
        self.b = b                      # batches
        self.s = s                      # seq per batch
        self.d = d                      # model dim
        self.cpc = cpc                  # projection cols per core (2 heads x 64)
        self.dk = dk                    # head dim
        self.seq = b * s                # total rows
        self.nkc = d // 128             # k chunks for projections
        self.sc = 512                   # s-chunk width (free dim of matmuls)
        self.nsc = self.seq // self.sc  # s chunks over the whole input
        self.ntb = s // 128             # t chunks per batch
        self.ntp = self.ntb // 2        # t-chunk pairs per batch
        self.nsb = s // self.sc         # s chunks per batch
        self.ntc_g = self.seq // 128    # global t chunks


def _build_nc(cfg: Cfg):
    c = cfg
    dk = c.dk
    nc = bacc.Bacc("TRN2", target_bir_lowering=False, debug=False,
                   num_devices=N_CORES)

    xT = nc.dram_tensor("xT", [c.d, c.seq], BF16, kind="ExternalInput").ap()
    wq = nc.dram_tensor("wq", [c.d, c.cpc], BF16, kind="ExternalInput").ap()
    wk = nc.dram_tensor("wk", [c.d, c.cpc], BF16, kind="ExternalInput").ap()
    wv = nc.dram_tensor("wv", [c.d, c.cpc], BF16, kind="ExternalInput").ap()
    wo = nc.dram_tensor("wo", [c.cpc, c.d], BF16, kind="ExternalInput").ap()
    bq = nc.dram_tensor("bq", [c.cpc, 1], F32, kind="ExternalInput").ap()
    bk = nc.dram_tensor("bk", [c.cpc, 1], F32, kind="ExternalInput").ap()
    out = nc.dram_tensor("out", [c.seq, c.d], F32, kind="ExternalOutput").ap()

    xT_r = xT.rearrange("(kc p) s -> p kc s", p=128)

    with tile.TileContext(nc) as tc:
        with tc.tile_pool(name="persist", bufs=1) as persist:
            qT_sb = persist.tile([128, c.seq], BF16, tag="qT")
            kT_sb = persist.tile([128, c.seq], BF16, tag="kT")
            # v in PV-lhsT layout, 128 cols per head per t-chunk:
            #   cols 0:64    = vA          (PV-A out partitions 0:64 = attnA)
            #   col 64       = ones        (PV-A out partition 64 = denomA)
            #   cols 65:128  = zeros
            #   col 128      = ones        (PV-B out partition 0 = denomB)
            #   cols 129:192 = zeros
            #   cols 192:256 = vB          (PV-B out partitions 64:128 = attnB)
            # so attnA lands at psum partitions 0:64 and attnB at 64:128,
            # letting the out-projection contract both heads in one K=128
            # matmul against the unsplit wo.
            v_sb = persist.tile([128, c.ntc_g, 4 * dk], BF16, tag="v")
            wq_sb = persist.tile([128, c.nkc, c.cpc], BF16, tag="wq")
            wk_sb = persist.tile([128, c.nkc, c.cpc], BF16, tag="wk")
            wv_sb = persist.tile([128, c.nkc, c.cpc], BF16, tag="wv")
            wo_sb = persist.tile([c.cpc, c.d], BF16, tag="wo")
            bq_sb = persist.tile([c.cpc, 1], F32, tag="bq")
            bk_sb = persist.tile([c.cpc, 1], F32, tag="bk")
            ones64 = persist.tile([1, dk], BF16, tag="ones64")

            nc.sync.dma_start(out=wq_sb[:], in_=wq.rearrange("(kc p) m -> p kc m", p=128))
            nc.sync.dma_start(out=wk_sb[:], in_=wk.rearrange("(kc p) m -> p kc m", p=128))
            nc.sync.dma_start(out=wv_sb[:], in_=wv.rearrange("(kc p) m -> p kc m", p=128))
            nc.sync.dma_start(out=wo_sb[:], in_=wo[:])
            nc.sync.dma_start(out=bq_sb[:], in_=bq[:])
            nc.sync.dma_start(out=bk_sb[:], in_=bk[:])

            ones_f32 = persist.tile([128, 1], F32, tag="ones_f32")
            nc.vector.memset(v_sb[:], 0.0)
            nc.vector.memset(ones_f32[:], 1.0)
            nc.vector.tensor_copy(ones64[:], ones_f32[0:1, :].to_broadcast([1, dk]))
            nc.vector.tensor_copy(
                v_sb[:, :, dk:dk + 1],
                ones_f32[:].unsqueeze(1).to_broadcast([128, c.ntc_g, 1]))
            nc.vector.tensor_copy(
                v_sb[:, :, 2 * dk:2 * dk + 1],
                ones_f32[:].unsqueeze(1).to_broadcast([128, c.ntc_g, 1]))

            with tc.tile_pool(name="xin", bufs=2) as xpool, \
                 tc.tile_pool(name="ps2", bufs=2, space="PSUM") as ps2, \
                 tc.tile_pool(name="pvp", bufs=1, space="PSUM") as pvp, \
                 tc.tile_pool(name="prj", bufs=2, space="PSUM") as prj, \
                 tc.tile_pool(name="exp", bufs=3) as epool, \
                 tc.tile_pool(name="norm", bufs=2) as npool, \
                 tc.tile_pool(name="rec", bufs=2) as rpool, \
                 tc.tile_pool(name="osb", bufs=3) as opool:

                xc_tiles = {}

                def dma_x(sc_i):
                    x_t = xpool.tile([128, c.nkc, c.sc], BF16, tag="x",
                                     name=f"xc_{sc_i}")
                    s0 = sc_i * c.sc
                    nc.sync.dma_start(out=x_t[:], in_=xT_r[:, :, s0:s0 + c.sc])
                    xc_tiles[sc_i] = x_t

                # Per-chunk projection state (psum tiles held across stages)
                pstate = {}

                def proj_stage(sc_i, stage):
                    """Emit 1/8th of projection chunk sc_i (stages 0..7)."""
                    xc = xc_tiles[sc_i]
                    s0 = sc_i * c.sc
                    st = pstate.setdefault(sc_i, {})
                    if stage == 0:
                        q_ps = prj.tile([128, c.sc], F32, tag="prj",
                                        name=f"q_ps_{sc_i}")
                        st["q"] = q_ps
                        for kc in range(4):
                            nc.tensor.matmul(q_ps[:], wq_sb[:, kc, :],
                                             xc[:, kc, :],
                                             start=(kc == 0), stop=False)
                    elif stage == 1:
                        q_ps = st.pop("q")
                        for kc in range(4, c.nkc):
                            nc.tensor.matmul(q_ps[:], wq_sb[:, kc, :],
                                             xc[:, kc, :],
                                             start=False, stop=(kc == c.nkc - 1))
                        nc.vector.tensor_scalar(
                            qT_sb[:, s0:s0 + c.sc], q_ps[:], bq_sb[:], None,
                            ALU.add)
                    elif stage == 2:
                        k_ps = prj.tile([128, c.sc], F32, tag="prj",
                                        name=f"k_ps_{sc_i}")
                        st["k"] = k_ps
                        for kc in range(4):
                            nc.tensor.matmul(k_ps[:], wk_sb[:, kc, :],
                                             xc[:, kc, :],
                                             start=(kc == 0), stop=False)
                    elif stage == 3:
                        k_ps = st.pop("k")
                        for kc in range(4, c.nkc):
                            nc.tensor.matmul(k_ps[:], wk_sb[:, kc, :],
                                             xc[:, kc, :],
                                             start=False, stop=(kc == c.nkc - 1))
                        nc.vector.tensor_scalar(
                            kT_sb[:, s0:s0 + c.sc], k_ps[:], bk_sb[:], None,
                            ALU.add)
                    elif stage in (4, 5, 6):
                        if stage == 4:
                            v_ps = prj.tile([128, 4, 128], F32, tag="prj",
                                            name=f"v_ps_{sc_i}")
                            st["v"] = v_ps
                        v_ps = st["v"]
                        # One accumulation group for the whole bank: start
                        # zeroes the full 2KB zero region, so only the very
                        # first matmul may set start and only the last stop.
                        for kc in range(2 * (stage - 4), 2 * (stage - 4) + 2):
                            for j in range(4):
                                nc.tensor.matmul(
                                    v_ps[:, j, :],
                                    xc[:, kc, j * 128:(j + 1) * 128],
                                    wv_sb[:, kc, :],
                                    start=(kc == 0 and j == 0), stop=False,
                                    skip_group_check=True)
                    else:  # stage 7
                        v_ps = st.pop("v")
                        for kc in (6, 7):
                            for j in range(4):
                                nc.tensor.matmul(
                                    v_ps[:, j, :],
                                    xc[:, kc, j * 128:(j + 1) * 128],
                                    wv_sb[:, kc, :],
                                    start=False, stop=(kc == 7 and j == 3),
                                    skip_group_check=True)
                        tc0 = sc_i * 4
                        for j in range(4):
                            nc.vector.tensor_copy(
                                v_sb[:, tc0 + j, 0:dk], v_ps[:, j, 0:dk])
                            nc.vector.tensor_copy(
                                v_sb[:, tc0 + j, 3 * dk:4 * dk],
                                v_ps[:, j, dk:2 * dk])
                        del xc_tiles[sc_i]
                        pstate.pop(sc_i, None)

                # ---------------- batch 0 projections upfront ----------------
                for sc_i in range(c.nsb):
                    dma_x(sc_i)
                for sc_i in range(c.nsb):
                    for stage in range(8):
                        proj_stage(sc_i, stage)
                # prefetch x for the first interleaved proj chunk
                if c.b > 1:
                    dma_x(c.nsb)

                # ---------------- main loop ----------------
                # part2 is software-pipelined: the DVE reciprocal chain for
                # chunk n is emitted right after its t-loop, but the PE part
                # (bcast matmul, norm, out-proj) is deferred into chunk n+1's
                # first t-iteration so the PE streams scores while DVE works.
                pending = {}

                def part2_dve(b_i, sb, pva, pvb):
                    den_a = rpool.tile([1, c.sc], F32, tag="den")
                    den_b = rpool.tile([1, c.sc], F32, tag="den")
                    # NB: reciprocal_approx_fast directly on the PSUM rows
                    # returns garbage on HW even though an isolated probe of
                    # the same AP works — stage denominators through SBUF.
                    nc.vector.tensor_copy(den_a[:], pva[dk:dk + 1, :])
                    nc.vector.tensor_copy(den_b[:], pvb[0:1, :])
                    rec_af = rpool.tile([1, c.sc], F32, tag="recf")
                    rec_bf = rpool.tile([1, c.sc], F32, tag="recf")
                    nc.vector.reciprocal_approx_fast(out=rec_af[:], in_=den_a[:])
                    nc.vector.reciprocal_approx_fast(out=rec_bf[:], in_=den_b[:])
                    rec_ab = rpool.tile([1, c.sc], BF16, tag="recb")
                    rec_bb = rpool.tile([1, c.sc], BF16, tag="recb")
                    nc.vector.tensor_copy(rec_ab[:], rec_af[:])
                    nc.vector.tensor_copy(rec_bb[:], rec_bf[:])
                    pending.update(b_i=b_i, sb=sb, pva=pva, pvb=pvb,
                                   rec_ab=rec_ab, rec_bb=rec_bb)

                def part2_pe():
                    if not pending:
                        return
                    b_i, sb = pending["b_i"], pending["sb"]
                    pva, pvb = pending["pva"], pending["pvb"]
                    s0 = b_i * c.s + sb * c.sc
                    bc2 = ps2.tile([128, 2 * c.sc], F32, tag="s2",
                                   name=f"bc2_{b_i}_{sb}")
                    nc.tensor.matmul(bc2[0:dk, 0:c.sc], ones64[:],
                                     pending["rec_ab"][:], start=True, stop=True)
                    nc.tensor.matmul(bc2[dk:2 * dk, c.sc:2 * c.sc], ones64[:],
                                     pending["rec_bb"][:], start=True, stop=True)
                    bcs = npool.tile([2 * dk, 2 * c.sc], F32, tag="bcs")
                    nc.vector.tensor_copy(bcs[0:dk, 0:c.sc],
                                          bc2[0:dk, 0:c.sc])
                    nc.vector.tensor_copy(bcs[dk:2 * dk, c.sc:2 * c.sc],
                                          bc2[dk:2 * dk, c.sc:2 * c.sc])
                    nab = npool.tile([2 * dk, c.sc], BF16, tag="nab")
                    nc.vector.tensor_tensor(
                        nab[0:dk, :], pva[0:dk, :], bcs[0:dk, 0:c.sc],
                        ALU.mult)
                    nc.vector.tensor_tensor(
                        nab[dk:2 * dk, :], pvb[dk:2 * dk, :],
                        bcs[dk:2 * dk, c.sc:2 * c.sc], ALU.mult)
                    for j in range(c.sc // 128):
                        o2 = ps2.tile([128, 2 * c.sc], F32, tag="s2",
                                      name=f"o2_{b_i}_{sb}_{j}")
                        for e in range(2):
                            nc.tensor.matmul(
                                o2[:, e * c.sc:(e + 1) * c.sc],
                                nab[:, j * 128:(j + 1) * 128],
                                wo_sb[:, e * c.sc:(e + 1) * c.sc],
                                start=True, stop=True)
                        osb = opool.tile([128, c.d], F32, tag="osb",
                                         name=f"osb_{b_i}_{sb}_{j}")
                        nc.vector.tensor_copy(osb[:], o2[:])
                        nc.sync.dma_start(
                            out=out[s0 + j * 128:s0 + (j + 1) * 128, :],
                            in_=osb[:])
                    pending.clear()

                for b_i in range(c.b):
                    for sb in range(c.nsb):
                        s0 = b_i * c.s + sb * c.sc
                        proj_c = (b_i + 1) * c.nsb + sb if b_i + 1 < c.b else None
                        pva = pvp.tile([128, c.sc], F32, tag="pva")
                        pvb = pvp.tile([128, c.sc], F32, tag="pvb")
                        for tp in range(c.ntp):
                            t0 = b_i * c.s + (2 * tp) * 128
                            t1 = t0 + 128
                            s2a = ps2.tile([128, 2 * c.sc], F32, tag="s2",
                                           name=f"s2a_{b_i}_{sb}_{tp}")
                            nc.tensor.matmul(
                                s2a[:, 0:c.sc], kT_sb[0:dk, t0:t0 + 128],
                                qT_sb[0:dk, s0:s0 + c.sc],
                                start=True, stop=True)
                            nc.tensor.matmul(
                                s2a[:, c.sc:2 * c.sc], kT_sb[0:dk, t1:t1 + 128],
                                qT_sb[0:dk, s0:s0 + c.sc],
                                start=True, stop=True)
                            e2a = epool.tile([128, 2 * c.sc], BF16, tag="e2",
                                             name=f"e2a_{b_i}_{sb}_{tp}")
                            nc.scalar.activation(e2a[:], s2a[:], AF.Exp,
                                                 scale=1.0 / np.sqrt(dk))
                            s2b = ps2.tile([128, 2 * c.sc], F32, tag="s2",
                                           name=f"s2b_{b_i}_{sb}_{tp}")
                            nc.tensor.matmul(
                                s2b[:, 0:c.sc], kT_sb[dk:2 * dk, t0:t0 + 128],
                                qT_sb[dk:2 * dk, s0:s0 + c.sc],
                                start=True, stop=True)
                            nc.tensor.matmul(
                                s2b[:, c.sc:2 * c.sc], kT_sb[dk:2 * dk, t1:t1 + 128],
                                qT_sb[dk:2 * dk, s0:s0 + c.sc],
                                start=True, stop=True)
                            e2b = epool.tile([128, 2 * c.sc], BF16, tag="e2",
                                             name=f"e2b_{b_i}_{sb}_{tp}")
                            nc.scalar.activation(e2b[:], s2b[:], AF.Exp,
                                                 scale=1.0 / np.sqrt(dk))
                            if tp == 0:
                                part2_pe()  # deferred from the previous chunk
                            tca = b_i * c.ntb + 2 * tp
                            tcb = tca + 1
                            nc.tensor.matmul(
                                pva[:], v_sb[:, tca, 0:2 * dk], e2a[:, 0:c.sc],
                                start=(tp == 0), stop=False)
                            nc.tensor.matmul(
                                pva[:], v_sb[:, tcb, 0:2 * dk],
                                e2a[:, c.sc:2 * c.sc],
                                start=False, stop=(tp == c.ntp - 1))
                            nc.tensor.matmul(
                                pvb[:], v_sb[:, tca, 2 * dk:4 * dk],
                                e2b[:, 0:c.sc],
                                start=(tp == 0), stop=False)
                            nc.tensor.matmul(
                                pvb[:], v_sb[:, tcb, 2 * dk:4 * dk],
                                e2b[:, c.sc:2 * c.sc],
                                start=False, stop=(tp == c.ntp - 1))
                            if proj_c is not None:
                                for st_i in range(tp * 8 // c.ntp,
                                                  (tp + 1) * 8 // c.ntp):
                                    proj_stage(proj_c, st_i)

                        part2_dve(b_i, sb, pva, pvb)

                        # prefetch x for the next interleaved proj chunk
                        if proj_c is not None:
                            nxt = proj_c + 1
                            if nxt < c.nsc and nxt not in xc_tiles:
                                dma_x(nxt)

                part2_pe()  # flush the final chunk

    nc.compile()
    return nc


_NC_CACHE = {}


def get_nc(cfg: Cfg | None = None):
    cfg = cfg or Cfg()
    key = (cfg.b, cfg.s, cfg.d, cfg.cpc, cfg.dk)
    if key not in _NC_CACHE:
        _NC_CACHE[key] = _build_nc(cfg)
    return _NC_CACHE[key]


def make_in_maps(x, w_q, b_q, w_k, b_k, w_v, b_v, w_o, b_o,
                 a_q, u_q, a_k, u_k, a_v, u_v, cfg: Cfg | None = None):
    """Host-side prep: merge LoRA, transpose x, cast to bf16, shard."""
    c = cfg or Cfg()
    x = np.asarray(x, np.float32)
    w_o = np.asarray(w_o, np.float32)

    def merge(w, a, u):
        return (np.asarray(w, np.float64)
                + (np.asarray(a, np.float64) @ np.asarray(u, np.float64))
                * SCALING).astype(np.float32)

    wq_eff = merge(w_q, a_q, u_q)
    wk_eff = merge(w_k, a_k, u_k)
    wv_eff = merge(w_v, a_v, u_v)

    xT = np.ascontiguousarray(x.reshape(c.seq, c.d).T).astype(NP_BF16)
    b_q = np.asarray(b_q, np.float32)
    b_k = np.asarray(b_k, np.float32)
    in_maps = []
    for i in range(N_CORES):
        sl = slice(i * c.cpc, (i + 1) * c.cpc)
        in_maps.append({
            "xT": xT,
            "wq": np.ascontiguousarray(wq_eff[:, sl]).astype(NP_BF16),
            "wk": np.ascontiguousarray(wk_eff[:, sl]).astype(NP_BF16),
            "wv": np.ascontiguousarray(wv_eff[:, sl]).astype(NP_BF16),
            "wo": np.ascontiguousarray(w_o[sl, :]).astype(NP_BF16),
            "bq": np.ascontiguousarray(b_q[sl]).reshape(c.cpc, 1),
            "bk": np.ascontiguousarray(b_k[sl]).reshape(c.cpc, 1),
        })
    return in_maps


def kernel(x, w_q, b_q, w_k, b_k, w_v, b_v, w_o, b_o,
           a_q, u_q, a_k, u_k, a_v, u_v):
    cfg = Cfg()
    c = cfg
    in_maps = make_in_maps(x, w_q, b_q, w_k, b_k, w_v, b_v, w_o, b_o,
                           a_q, u_q, a_k, u_k, a_v, u_v, cfg)
    nc = get_nc(cfg)
    res = run_bass_kernel_spmd(nc, in_maps, list(range(N_CORES)))
    out = np.zeros((c.seq, c.d), np.float32)
    for i in range(N_CORES):
        out += res.results[i]["out"]
    # v-bias rides through softmax as a constant row; b_o is plain bias
    b_v = np.asarray(b_v, np.float32)
    b_o = np.asarray(b_o, np.float32)
    w_o = np.asarray(w_o, np.float32)
    out += (b_v @ w_o + b_o).astype(np.float32)
    return out.reshape(B, S, D_MODEL).astype(np.float32)


# revision 21
# speedup vs baseline: 2.3067x; 1.0888x over previous
"""LoRA attention Bass kernel for 8x Trainium2 NeuronCores.

Sharding (Megatron tensor-parallel over heads):
  - Each of the 8 cores owns 2 heads (128 projection columns).
  - q/k/v projections column-sharded; out projection row-sharded;
    per-core partial outputs are summed on the host.
  - LoRA is merged into the base weights on the host (w_eff = w + a@u*scaling),
    which is exact up to fp32 rounding.

All matmul operands are bf16 (fp32/f32r matmuls trip the PE power throttle
to a 50% duty cycle; bf16 streams at 1 row/cycle at 2.4 GHz). PSUM
accumulation stays fp32.

Device schedule (per core):
  Phase 1 (proj): qT/kT computed transposed ([proj_col, seq]) from xT tiles;
      v computed in natural layout ([seq, proj_col]). Biases folded in via
      DVE tensor_scalar on the PSUM->SBUF copy. Projections for batch b+1 are
      interleaved into batch b's attention t-loop to fill PE idle slots
      (the t-loop is ACT-bound).
  Phase 2 (attention): S^T = K @ Q^T per (batch, head); scores for two
      t-chunks share one 2-bank PSUM tile so a single ACT exp instruction
      covers 1024 elements (halves ACT per-instruction overhead). P@V uses
      lhsT=[v | ones] so the softmax denominator falls out of the same
      matmul (row 64 of the PSUM output).
  Phase 3 (out-proj): recip via DVE reciprocal_approx_fast, broadcast to 64
      partitions via a K=1 ones matmul, normalize+cast to bf16 on DVE, then
      out = attnout @ Wo_slice accumulated in PSUM and DMA'd out.

PSUM budget (8 banks): s2 tag 2x[128,1024] (scores / bcast / out-proj) = 4,
  pv_a + pv_b = 2, proj accumulators (q/k/v rotating, bufs=2) = 2.
"""

import numpy as np
import ml_dtypes

import concourse.bass as bass
import concourse.mybir as mybir
import concourse.tile as tile
from concourse import bacc
from concourse.bass_utils import run_bass_kernel_spmd

F32 = mybir.dt.float32
F32R = mybir.dt.float32r
BF16 = mybir.dt.bfloat16
AF = mybir.ActivationFunctionType
ALU = mybir.AluOpType

N_CORES = 8

# Full-problem dims (hardcoded per spec)
D_MODEL = 1024
N_HEADS = 16
D_K = 64
LORA_R = 8
SCALING = 2.0
B = 4
S = 2048

NP_BF16 = ml_dtypes.bfloat16


class Cfg:
    def __init__(self, b=B, s=S, d=D_MODEL, cpc=128, dk=D_K):
        self.b = b                      # batches
        self.s = s                      # seq per batch
        self.d = d                      # model dim
        self.cpc = cpc                  # projection cols per core (2 heads x 64)
        self.dk = dk                    # head dim
        self.seq = b * s                # total rows
        self.nkc = d // 128             # k chunks for projections
        self.sc = 512                   # s-chunk width (free dim of matmuls)
        self.nsc = self.seq // self.sc  # s chunks over the whole input
        self.ntb = s // 128             # t chunks per batch
        self.ntp = self.ntb // 2        # t-chunk pairs per batch
        self.nsb = s // self.sc         # s chunks per batch
        self.ntc_g = self.seq // 128    # global t chunks


def _build_nc(cfg: Cfg):
    c = cfg
    dk = c.dk
    nc = bacc.Bacc("TRN2", target_bir_lowering=False, debug=False,
                   num_devices=N_CORES)

    xT = nc.dram_tensor("xT", [c.d, c.seq], BF16, kind="ExternalInput").ap()
    wq = nc.dram_tensor("wq", [c.d, c.cpc], BF16, kind="ExternalInput").ap()
    wk = nc.dram_tensor("wk", [c.d, c.cpc], BF16, kind="ExternalInput").ap()
    wv = nc.dram_tensor("wv", [c.d, c.cpc], BF16, kind="ExternalInput").ap()
    wo = nc.dram_tensor("wo", [c.cpc, c.d], BF16, kind="ExternalInput").ap()
    bq = nc.dram_tensor("bq", [c.cpc, 1], F32, kind="ExternalInput").ap()
    bk = nc.dram_tensor("bk", [c.cpc, 1], F32, kind="ExternalInput").ap()
    out = nc.dram_tensor("out", [c.seq, c.d], F32, kind="ExternalOutput").ap()

    xT_r = xT.rearrange("(kc p) s -> p kc s", p=128)

    with tile.TileContext(nc) as tc:
        with tc.tile_pool(name="persist", bufs=1) as persist:
            qT_sb = persist.tile([128, c.seq], BF16, tag="qT")
            kT_sb = persist.tile([128, c.seq], BF16, tag="kT")
            # v in PV-lhsT layout, 128 cols per head per t-chunk:
            #   cols 0:64    = vA          (PV-A out partitions 0:64 = attnA)
            #   col 64       = ones        (PV-A out partition 64 = denomA)
            #   cols 65:128  = zeros
            #   col 128      = ones        (PV-B out partition 0 = denomB)
            #   cols 129:192 = zeros
            #   cols 192:256 = vB          (PV-B out partitions 64:128 = attnB)
            # so attnA lands at psum partitions 0:64 and attnB at 64:128,
            # letting the out-projection contract both heads in one K=128
            # matmul against the unsplit wo.
            v_sb = persist.tile([128, c.ntc_g, 4 * dk], BF16, tag="v")
            wq_sb = persist.tile([128, c.nkc, c.cpc], BF16, tag="wq")
            wk_sb = persist.tile([128, c.nkc, c.cpc], BF16, tag="wk")
            wv_sb = persist.tile([128, c.nkc, c.cpc], BF16, tag="wv")
            wo_sb = persist.tile([c.cpc, c.d], BF16, tag="wo")
            bq_sb = persist.tile([c.cpc, 1], F32, tag="bq")
            bk_sb = persist.tile([c.cpc, 1], F32, tag="bk")
            ones64 = persist.tile([1, dk], BF16, tag="ones64")

            nc.sync.dma_start(out=wq_sb[:], in_=wq.rearrange("(kc p) m -> p kc m", p=128))
            nc.sync.dma_start(out=wk_sb[:], in_=wk.rearrange("(kc p) m -> p kc m", p=128))
            nc.sync.dma_start(out=wv_sb[:], in_=wv.rearrange("(kc p) m -> p kc m", p=128))
            nc.sync.dma_start(out=wo_sb[:], in_=wo[:])
            nc.sync.dma_start(out=bq_sb[:], in_=bq[:])
            nc.sync.dma_start(out=bk_sb[:], in_=bk[:])

            ones_f32 = persist.tile([128, 1], F32, tag="ones_f32")
            nc.vector.memset(v_sb[:], 0.0)
            nc.vector.memset(ones_f32[:], 1.0)
            nc.vector.tensor_copy(ones64[:], ones_f32[0:1, :].to_broadcast([1, dk]))
            nc.vector.tensor_copy(
                v_sb[:, :, dk:dk + 1],
                ones_f32[:].unsqueeze(1).to_broadcast([128, c.ntc_g, 1]))
            nc.vector.tensor_copy(
                v_sb[:, :, 2 * dk:2 * dk + 1],
                ones_f32[:].unsqueeze(1).to_broadcast([128, c.ntc_g, 1]))

            with tc.tile_pool(name="xin", bufs=3) as xpool, \
                 tc.tile_pool(name="ps2", bufs=2, space="PSUM") as ps2, \
                 tc.tile_pool(name="pvp", bufs=1, space="PSUM") as pvp, \
                 tc.tile_pool(name="prj", bufs=2, space="PSUM") as prj, \
                 tc.tile_pool(name="exp", bufs=3) as epool, \
                 tc.tile_pool(name="norm", bufs=2) as npool, \
                 tc.tile_pool(name="rec", bufs=2) as rpool, \
                 tc.tile_pool(name="osb", bufs=3) as opool:

                xc_tiles = {}

                def dma_x(sc_i):
                    x_t = xpool.tile([128, c.nkc, c.sc], BF16, tag="x",
                                     name=f"xc_{sc_i}")
                    s0 = sc_i * c.sc
                    nc.sync.dma_start(out=x_t[:], in_=xT_r[:, :, s0:s0 + c.sc])
                    xc_tiles[sc_i] = x_t

                # Per-chunk projection state (psum tiles held across stages)
                pstate = {}

                def proj_stage(sc_i, stage):
                    """Emit 1/8th of projection chunk sc_i (stages 0..7)."""
                    xc = xc_tiles[sc_i]
                    s0 = sc_i * c.sc
                    st = pstate.setdefault(sc_i, {})
                    if stage == 0:
                        q_ps = prj.tile([128, c.sc], F32, tag="prj",
                                        name=f"q_ps_{sc_i}")
                        st["q"] = q_ps
                        for kc in range(4):
                            nc.tensor.matmul(q_ps[:], wq_sb[:, kc, :],
                                             xc[:, kc, :],
                                             start=(kc == 0), stop=False)
                    elif stage == 1:
                        q_ps = st.pop("q")
                        for kc in range(4, c.nkc):
                            nc.tensor.matmul(q_ps[:], wq_sb[:, kc, :],
                                             xc[:, kc, :],
                                             start=False, stop=(kc == c.nkc - 1))
                        nc.vector.tensor_scalar(
                            qT_sb[:, s0:s0 + c.sc], q_ps[:], bq_sb[:], None,
                            ALU.add)
                    elif stage == 2:
                        k_ps = prj.tile([128, c.sc], F32, tag="prj",
                                        name=f"k_ps_{sc_i}")
                        st["k"] = k_ps
                        for kc in range(4):
                            nc.tensor.matmul(k_ps[:], wk_sb[:, kc, :],
                                             xc[:, kc, :],
                                             start=(kc == 0), stop=False)
                    elif stage == 3:
                        k_ps = st.pop("k")
                        for kc in range(4, c.nkc):
                            nc.tensor.matmul(k_ps[:], wk_sb[:, kc, :],
                                             xc[:, kc, :],
                                             start=False, stop=(kc == c.nkc - 1))
                        nc.vector.tensor_scalar(
                            kT_sb[:, s0:s0 + c.sc], k_ps[:], bk_sb[:], None,
                            ALU.add)
                    elif stage in (4, 5, 6):
                        if stage == 4:
                            v_ps = prj.tile([128, 4, 128], F32, tag="prj",
                                            name=f"v_ps_{sc_i}")
                            st["v"] = v_ps
                        v_ps = st["v"]
                        # One accumulation group for the whole bank: start
                        # zeroes the full 2KB zero region, so only the very
                        # first matmul may set start and only the last stop.
                        for kc in range(2 * (stage - 4), 2 * (stage - 4) + 2):
                            for j in range(4):
                                nc.tensor.matmul(
                                    v_ps[:, j, :],
                                    xc[:, kc, j * 128:(j + 1) * 128],
                                    wv_sb[:, kc, :],
                                    start=(kc == 0 and j == 0), stop=False,
                                    skip_group_check=True)
                    else:  # stage 7
                        v_ps = st.pop("v")
                        for kc in (6, 7):
                            for j in range(4):
                                nc.tensor.matmul(
                                    v_ps[:, j, :],
                                    xc[:, kc, j * 128:(j + 1) * 128],
                                    wv_sb[:, kc, :],
                                    start=False, stop=(kc == 7 and j == 3),
                                    skip_group_check=True)
                        tc0 = sc_i * 4
                        for j in range(4):
                            nc.vector.tensor_copy(
                                v_sb[:, tc0 + j, 0:dk], v_ps[:, j, 0:dk])
                            nc.vector.tensor_copy(
                                v_sb[:, tc0 + j, 3 * dk:4 * dk],
                                v_ps[:, j, dk:2 * dk])
                        del xc_tiles[sc_i]
                        pstate.pop(sc_i, None)

                # ---------------- batch 0 projections upfront ----------------
                for sc_i in range(c.nsb):
                    dma_x(sc_i)
                for sc_i in range(c.nsb):
                    for stage in range(8):
                        proj_stage(sc_i, stage)
                # prefetch x for the first interleaved proj chunk
                if c.b > 1:
                    dma_x(c.nsb)

                # ---------------- main loop ----------------
                # part2 is software-pipelined: the DVE reciprocal chain for
                # chunk n is emitted right after its t-loop, but the PE part
                # (bcast matmul, norm, out-proj) is deferred into chunk n+1's
                # first t-iteration so the PE streams scores while DVE works.
                pending = {}

                def part2_dve(b_i, sb, pva, pvb):
                    den_a = rpool.tile([1, c.sc], F32, tag="den")
                    den_b = rpool.tile([1, c.sc], F32, tag="den")
                    # NB: reciprocal_approx_fast directly on the PSUM rows
                    # returns garbage on HW even though an isolated probe of
                    # the same AP works — stage denominators through SBUF.
                    nc.vector.tensor_copy(den_a[:], pva[dk:dk + 1, :])
                    nc.vector.tensor_copy(den_b[:], pvb[0:1, :])
                    rec_af = rpool.tile([1, c.sc], F32, tag="recf")
                    rec_bf = rpool.tile([1, c.sc], F32, tag="recf")
                    nc.vector.reciprocal_approx_fast(out=rec_af[:], in_=den_a[:])
                    nc.vector.reciprocal_approx_fast(out=rec_bf[:], in_=den_b[:])
                    rec_ab = rpool.tile([1, c.sc], BF16, tag="recb")
                    rec_bb = rpool.tile([1, c.sc], BF16, tag="recb")
                    nc.vector.tensor_copy(rec_ab[:], rec_af[:])
                    nc.vector.tensor_copy(rec_bb[:], rec_bf[:])
                    pending.update(b_i=b_i, sb=sb, pva=pva, pvb=pvb,
                                   rec_ab=rec_ab, rec_bb=rec_bb)

                def part2_bc():
                    """Broadcast 1/denom to 64 partitions + normalize (nab)."""
                    if not pending:
                        return
                    b_i, sb = pending["b_i"], pending["sb"]
                    pva, pvb = pending["pva"], pending["pvb"]
                    bc2 = ps2.tile([128, 2 * c.sc], F32, tag="s2",
                                   name=f"bc2_{b_i}_{sb}")
                    nc.tensor.matmul(bc2[0:dk, 0:c.sc], ones64[:],
                                     pending["rec_ab"][:], start=True, stop=True)
                    nc.tensor.matmul(bc2[dk:2 * dk, c.sc:2 * c.sc], ones64[:],
                                     pending["rec_bb"][:], start=True, stop=True)
                    bcs = npool.tile([2 * dk, 2 * c.sc], F32, tag="bcs")
                    nc.scalar.copy(bcs[0:dk, 0:c.sc], bc2[0:dk, 0:c.sc])
                    nc.vector.tensor_copy(bcs[dk:2 * dk, c.sc:2 * c.sc],
                                          bc2[dk:2 * dk, c.sc:2 * c.sc])
                    nab = npool.tile([2 * dk, c.sc], BF16, tag="nab")
                    nc.vector.tensor_tensor(
                        nab[0:dk, :], pva[0:dk, :], bcs[0:dk, 0:c.sc],
                        ALU.mult)
                    nc.vector.tensor_tensor(
                        nab[dk:2 * dk, :], pvb[dk:2 * dk, :],
                        bcs[dk:2 * dk, c.sc:2 * c.sc], ALU.mult)
                    pending["nab"] = nab

                def part2_out():
                    if not pending:
                        return
                    b_i, sb = pending["b_i"], pending["sb"]
                    nab = pending["nab"]
                    s0 = b_i * c.s + sb * c.sc
                    for j in range(c.sc // 128):
                        o2 = ps2.tile([128, 2 * c.sc], F32, tag="s2",
                                      name=f"o2_{b_i}_{sb}_{j}")
                        for e in range(2):
                            nc.tensor.matmul(
                                o2[:, e * c.sc:(e + 1) * c.sc],
                                nab[:, j * 128:(j + 1) * 128],
                                wo_sb[:, e * c.sc:(e + 1) * c.sc],
                                start=True, stop=True)
                        osb = opool.tile([128, c.d], F32, tag="osb",
                                         name=f"osb_{b_i}_{sb}_{j}")
                        if j % 2 == 0:
                            nc.scalar.copy(osb[:], o2[:])
                        else:
                            nc.vector.tensor_copy(osb[:], o2[:])
                        nc.sync.dma_start(
                            out=out[s0 + j * 128:s0 + (j + 1) * 128, :],
                            in_=osb[:])
                    pending.clear()

                for b_i in range(c.b):
                    for sb in range(c.nsb):
                        s0 = b_i * c.s + sb * c.sc
                        proj_c = (b_i + 1) * c.nsb + sb if b_i + 1 < c.b else None
                        pva = pvp.tile([128, c.sc], F32, tag="pva")
                        pvb = pvp.tile([128, c.sc], F32, tag="pvb")
                        for tp in range(c.ntp):
                            t0 = b_i * c.s + (2 * tp) * 128
                            t1 = t0 + 128
                            s2a = ps2.tile([128, 2 * c.sc], F32, tag="s2",
                                           name=f"s2a_{b_i}_{sb}_{tp}")
                            nc.tensor.matmul(
                                s2a[:, 0:c.sc], kT_sb[0:dk, t0:t0 + 128],
                                qT_sb[0:dk, s0:s0 + c.sc],
                                start=True, stop=True)
                            nc.tensor.matmul(
                                s2a[:, c.sc:2 * c.sc], kT_sb[0:dk, t1:t1 + 128],
                                qT_sb[0:dk, s0:s0 + c.sc],
                                start=True, stop=True)
                            e2a = epool.tile([128, 2 * c.sc], BF16, tag="e2",
                                             name=f"e2a_{b_i}_{sb}_{tp}")
                            nc.scalar.activation(e2a[:], s2a[:], AF.Exp,
                                                 scale=1.0 / np.sqrt(dk))
                            s2b = ps2.tile([128, 2 * c.sc], F32, tag="s2",
                                           name=f"s2b_{b_i}_{sb}_{tp}")
                            nc.tensor.matmul(
                                s2b[:, 0:c.sc], kT_sb[dk:2 * dk, t0:t0 + 128],
                                qT_sb[dk:2 * dk, s0:s0 + c.sc],
                                start=True, stop=True)
                            nc.tensor.matmul(
                                s2b[:, c.sc:2 * c.sc], kT_sb[dk:2 * dk, t1:t1 + 128],
                                qT_sb[dk:2 * dk, s0:s0 + c.sc],
                                start=True, stop=True)
                            e2b = epool.tile([128, 2 * c.sc], BF16, tag="e2",
                                             name=f"e2b_{b_i}_{sb}_{tp}")
                            nc.scalar.activation(e2b[:], s2b[:], AF.Exp,
                                                 scale=1.0 / np.sqrt(dk))
                            if tp == 0:
                                part2_bc()  # deferred from the previous chunk
                            elif tp == 1:
                                part2_out()
                            tca = b_i * c.ntb + 2 * tp
                            tcb = tca + 1
                            nc.tensor.matmul(
                                pva[:], v_sb[:, tca, 0:2 * dk], e2a[:, 0:c.sc],
                                start=(tp == 0), stop=False)
                            nc.tensor.matmul(
                                pva[:], v_sb[:, tcb, 0:2 * dk],
                                e2a[:, c.sc:2 * c.sc],
                                start=False, stop=(tp == c.ntp - 1))
                            nc.tensor.matmul(
                                pvb[:], v_sb[:, tca, 2 * dk:4 * dk],
                                e2b[:, 0:c.sc],
                                start=(tp == 0), stop=False)
                            nc.tensor.matmul(
                                pvb[:], v_sb[:, tcb, 2 * dk:4 * dk],
                                e2b[:, c.sc:2 * c.sc],
                                start=False, stop=(tp == c.ntp - 1))
                            if proj_c is not None:
                                for st_i in range(tp * 8 // c.ntp,
                                                  (tp + 1) * 8 // c.ntp):
                                    proj_stage(proj_c, st_i)

                        part2_dve(b_i, sb, pva, pvb)

                        # prefetch x for the next interleaved proj chunk
                        if proj_c is not None:
                            nxt = proj_c + 1
                            if nxt < c.nsc and nxt not in xc_tiles:
                                dma_x(nxt)

                part2_bc()  # flush the final chunk
                part2_out()

    nc.compile()
    return nc


_NC_CACHE = {}


def get_nc(cfg: Cfg | None = None):
    cfg = cfg or Cfg()
    key = (cfg.b, cfg.s, cfg.d, cfg.cpc, cfg.dk)
    if key not in _NC_CACHE:
        _NC_CACHE[key] = _build_nc(cfg)
    return _NC_CACHE[key]


def make_in_maps(x, w_q, b_q, w_k, b_k, w_v, b_v, w_o, b_o,
                 a_q, u_q, a_k, u_k, a_v, u_v, cfg: Cfg | None = None):
    """Host-side prep: merge LoRA, transpose x, cast to bf16, shard."""
    c = cfg or Cfg()
    x = np.asarray(x, np.float32)
    w_o = np.asarray(w_o, np.float32)

    def merge(w, a, u):
        return (np.asarray(w, np.float64)
                + (np.asarray(a, np.float64) @ np.asarray(u, np.float64))
                * SCALING).astype(np.float32)

    wq_eff = merge(w_q, a_q, u_q)
    wk_eff = merge(w_k, a_k, u_k)
    wv_eff = merge(w_v, a_v, u_v)

    xT = np.ascontiguousarray(x.reshape(c.seq, c.d).T).astype(NP_BF16)
    b_q = np.asarray(b_q, np.float32)
    b_k = np.asarray(b_k, np.float32)
    in_maps = []
    for i in range(N_CORES):
        sl = slice(i * c.cpc, (i + 1) * c.cpc)
        in_maps.append({
            "xT": xT,
            "wq": np.ascontiguousarray(wq_eff[:, sl]).astype(NP_BF16),
            "wk": np.ascontiguousarray(wk_eff[:, sl]).astype(NP_BF16),
            "wv": np.ascontiguousarray(wv_eff[:, sl]).astype(NP_BF16),
            "wo": np.ascontiguousarray(w_o[sl, :]).astype(NP_BF16),
            "bq": np.ascontiguousarray(b_q[sl]).reshape(c.cpc, 1),
            "bk": np.ascontiguousarray(b_k[sl]).reshape(c.cpc, 1),
        })
    return in_maps


def kernel(x, w_q, b_q, w_k, b_k, w_v, b_v, w_o, b_o,
           a_q, u_q, a_k, u_k, a_v, u_v):
    cfg = Cfg()
    c = cfg
    in_maps = make_in_maps(x, w_q, b_q, w_k, b_k, w_v, b_v, w_o, b_o,
                           a_q, u_q, a_k, u_k, a_v, u_v, cfg)
    nc = get_nc(cfg)
    res = run_bass_kernel_spmd(nc, in_maps, list(range(N_CORES)))
    out = np.zeros((c.seq, c.d), np.float32)
    for i in range(N_CORES):
        out += res.results[i]["out"]
    # v-bias rides through softmax as a constant row; b_o is plain bias
    b_v = np.asarray(b_v, np.float32)
    b_o = np.asarray(b_o, np.float32)
    w_o = np.asarray(w_o, np.float32)
    out += (b_v @ w_o + b_o).astype(np.float32)
    return out.reshape(B, S, D_MODEL).astype(np.float32)


# revision 23
# speedup vs baseline: 2.3156x; 1.0039x over previous
"""LoRA attention Bass kernel for 8x Trainium2 NeuronCores.

Sharding (Megatron tensor-parallel over heads):
  - Each of the 8 cores owns 2 heads (128 projection columns).
  - q/k/v projections column-sharded; out projection row-sharded;
    per-core partial outputs are summed on the host.
  - LoRA is merged into the base weights on the host (w_eff = w + a@u*scaling),
    which is exact up to fp32 rounding.

All matmul operands are bf16 (fp32/f32r matmuls trip the PE power throttle
to a 50% duty cycle; bf16 streams at 1 row/cycle at 2.4 GHz). PSUM
accumulation stays fp32.

Device schedule (per core):
  Phase 1 (proj): qT/kT computed transposed ([proj_col, seq]) from xT tiles;
      v computed in natural layout ([seq, proj_col]). Biases folded in via
      DVE tensor_scalar on the PSUM->SBUF copy. Projections for batch b+1 are
      interleaved into batch b's attention t-loop to fill PE idle slots
      (the t-loop is ACT-bound).
  Phase 2 (attention): S^T = K @ Q^T per (batch, head); scores for two
      t-chunks share one 2-bank PSUM tile so a single ACT exp instruction
      covers 1024 elements (halves ACT per-instruction overhead). P@V uses
      lhsT=[v | ones] so the softmax denominator falls out of the same
      matmul (row 64 of the PSUM output).
  Phase 3 (out-proj): recip via DVE reciprocal_approx_fast, broadcast to 64
      partitions via a K=1 ones matmul, normalize+cast to bf16 on DVE, then
      out = attnout @ Wo_slice accumulated in PSUM and DMA'd out.

PSUM budget (8 banks): s2 tag 2x[128,1024] (scores / bcast / out-proj) = 4,
  pv_a + pv_b = 2, proj accumulators (q/k/v rotating, bufs=2) = 2.
"""

import numpy as np
import ml_dtypes

import concourse.bass as bass
import concourse.mybir as mybir
import concourse.tile as tile
from concourse import bacc
from concourse.bass_utils import run_bass_kernel_spmd

F32 = mybir.dt.float32
F32R = mybir.dt.float32r
BF16 = mybir.dt.bfloat16
AF = mybir.ActivationFunctionType
ALU = mybir.AluOpType

N_CORES = 8

# Full-problem dims (hardcoded per spec)
D_MODEL = 1024
N_HEADS = 16
D_K = 64
LORA_R = 8
SCALING = 2.0
B = 4
S = 2048

NP_BF16 = ml_dtypes.bfloat16


class Cfg:
    def __init__(self, b=B, s=S, d=D_MODEL, cpc=128, dk=D_K):
        self.b = b                      # batches
        self.s = s                      # seq per batch
        self.d = d                      # model dim
        self.cpc = cpc                  # projection cols per core (2 heads x 64)
        self.dk = dk                    # head dim
        self.seq = b * s                # total rows
        self.nkc = d // 128             # k chunks for projections
        self.sc = 512                   # s-chunk width (free dim of matmuls)
        self.nsc = self.seq // self.sc  # s chunks over the whole input
        self.ntb = s // 128             # t chunks per batch
        self.ntp = self.ntb // 2        # t-chunk pairs per batch
        self.nsb = s // self.sc         # s chunks per batch
        self.ntc_g = self.seq // 128    # global t chunks


def _build_nc(cfg: Cfg):
    c = cfg
    dk = c.dk
    nc = bacc.Bacc("TRN2", target_bir_lowering=False, debug=False,
                   num_devices=N_CORES)

    xT = nc.dram_tensor("xT", [c.d, c.seq], BF16, kind="ExternalInput").ap()
    wq = nc.dram_tensor("wq", [c.d, c.cpc], BF16, kind="ExternalInput").ap()
    wk = nc.dram_tensor("wk", [c.d, c.cpc], BF16, kind="ExternalInput").ap()
    wv = nc.dram_tensor("wv", [c.d, c.cpc], BF16, kind="ExternalInput").ap()
    wo = nc.dram_tensor("wo", [c.cpc, c.d], BF16, kind="ExternalInput").ap()
    bq = nc.dram_tensor("bq", [c.cpc, 1], F32, kind="ExternalInput").ap()
    bk = nc.dram_tensor("bk", [c.cpc, 1], F32, kind="ExternalInput").ap()
    out = nc.dram_tensor("out", [c.seq, c.d], F32, kind="ExternalOutput").ap()

    xT_r = xT.rearrange("(kc p) s -> p kc s", p=128)

    with tile.TileContext(nc) as tc:
        with tc.tile_pool(name="persist", bufs=1) as persist:
            qT_sb = persist.tile([128, c.seq], BF16, tag="qT")
            kT_sb = persist.tile([128, c.seq], BF16, tag="kT")
            # v in PV-lhsT layout, 128 cols per head per t-chunk:
            #   cols 0:64    = vA          (PV-A out partitions 0:64 = attnA)
            #   col 64       = ones        (PV-A out partition 64 = denomA)
            #   cols 65:128  = zeros
            #   col 128      = ones        (PV-B out partition 0 = denomB)
            #   cols 129:192 = zeros
            #   cols 192:256 = vB          (PV-B out partitions 64:128 = attnB)
            # so attnA lands at psum partitions 0:64 and attnB at 64:128,
            # letting the out-projection contract both heads in one K=128
            # matmul against the unsplit wo.
            v_sb = persist.tile([128, c.ntc_g, 4 * dk], BF16, tag="v")
            wq_sb = persist.tile([128, c.nkc, c.cpc], BF16, tag="wq")
            wk_sb = persist.tile([128, c.nkc, c.cpc], BF16, tag="wk")
            wv_sb = persist.tile([128, c.nkc, c.cpc], BF16, tag="wv")
            wo_sb = persist.tile([c.cpc, c.d], BF16, tag="wo")
            bq_sb = persist.tile([c.cpc, 1], F32, tag="bq")
            bk_sb = persist.tile([c.cpc, 1], F32, tag="bk")
            ones64 = persist.tile([1, dk], BF16, tag="ones64")

            nc.sync.dma_start(out=wq_sb[:], in_=wq.rearrange("(kc p) m -> p kc m", p=128))
            nc.sync.dma_start(out=wk_sb[:], in_=wk.rearrange("(kc p) m -> p kc m", p=128))
            nc.sync.dma_start(out=wv_sb[:], in_=wv.rearrange("(kc p) m -> p kc m", p=128))
            nc.sync.dma_start(out=wo_sb[:], in_=wo[:])
            nc.sync.dma_start(out=bq_sb[:], in_=bq[:])
            nc.sync.dma_start(out=bk_sb[:], in_=bk[:])

            ones_f32 = persist.tile([128, 1], F32, tag="ones_f32")
            nc.vector.memset(v_sb[:], 0.0)
            nc.vector.memset(ones_f32[:], 1.0)
            nc.vector.tensor_copy(ones64[:], ones_f32[0:1, :].to_broadcast([1, dk]))
            nc.vector.tensor_copy(
                v_sb[:, :, dk:dk + 1],
                ones_f32[:].unsqueeze(1).to_broadcast([128, c.ntc_g, 1]))
            nc.vector.tensor_copy(
                v_sb[:, :, 2 * dk:2 * dk + 1],
                ones_f32[:].unsqueeze(1).to_broadcast([128, c.ntc_g, 1]))

            with tc.tile_pool(name="xin", bufs=3) as xpool, \
                 tc.tile_pool(name="ps2", bufs=2, space="PSUM") as ps2, \
                 tc.tile_pool(name="pvp", bufs=1, space="PSUM") as pvp, \
                 tc.tile_pool(name="prj", bufs=2, space="PSUM") as prj, \
                 tc.tile_pool(name="exp", bufs=3) as epool, \
                 tc.tile_pool(name="norm", bufs=2) as npool, \
                 tc.tile_pool(name="rec", bufs=2) as rpool, \
                 tc.tile_pool(name="osb", bufs=3) as opool:

                xc_tiles = {}

                def dma_x(sc_i):
                    x_t = xpool.tile([128, c.nkc, c.sc], BF16, tag="x",
                                     name=f"xc_{sc_i}")
                    s0 = sc_i * c.sc
                    nc.sync.dma_start(out=x_t[:], in_=xT_r[:, :, s0:s0 + c.sc])
                    xc_tiles[sc_i] = x_t

                # Per-chunk projection state (psum tiles held across stages)
                pstate = {}

                def proj_stage(sc_i, stage):
                    """Emit 1/8th of projection chunk sc_i (stages 0..7)."""
                    xc = xc_tiles[sc_i]
                    s0 = sc_i * c.sc
                    st = pstate.setdefault(sc_i, {})
                    if stage == 0:
                        q_ps = prj.tile([128, c.sc], F32, tag="prj",
                                        name=f"q_ps_{sc_i}")
                        st["q"] = q_ps
                        for kc in range(4):
                            nc.tensor.matmul(q_ps[:], wq_sb[:, kc, :],
                                             xc[:, kc, :],
                                             start=(kc == 0), stop=False)
                    elif stage == 1:
                        q_ps = st.pop("q")
                        for kc in range(4, c.nkc):
                            nc.tensor.matmul(q_ps[:], wq_sb[:, kc, :],
                                             xc[:, kc, :],
                                             start=False, stop=(kc == c.nkc - 1))
                        nc.vector.tensor_scalar(
                            qT_sb[:, s0:s0 + c.sc], q_ps[:], bq_sb[:], None,
                            ALU.add)
                    elif stage == 2:
                        k_ps = prj.tile([128, c.sc], F32, tag="prj",
                                        name=f"k_ps_{sc_i}")
                        st["k"] = k_ps
                        for kc in range(4):
                            nc.tensor.matmul(k_ps[:], wk_sb[:, kc, :],
                                             xc[:, kc, :],
                                             start=(kc == 0), stop=False)
                    elif stage == 3:
                        k_ps = st.pop("k")
                        for kc in range(4, c.nkc):
                            nc.tensor.matmul(k_ps[:], wk_sb[:, kc, :],
                                             xc[:, kc, :],
                                             start=False, stop=(kc == c.nkc - 1))
                        nc.vector.tensor_scalar(
                            kT_sb[:, s0:s0 + c.sc], k_ps[:], bk_sb[:], None,
                            ALU.add)
                    elif stage in (4, 5, 6):
                        if stage == 4:
                            v_ps = prj.tile([128, 4, 128], F32, tag="prj",
                                            name=f"v_ps_{sc_i}")
                            st["v"] = v_ps
                        v_ps = st["v"]
                        # One accumulation group for the whole bank: start
                        # zeroes the full 2KB zero region, so only the very
                        # first matmul may set start and only the last stop.
                        for kc in range(2 * (stage - 4), 2 * (stage - 4) + 2):
                            for j in range(4):
                                nc.tensor.matmul(
                                    v_ps[:, j, :],
                                    xc[:, kc, j * 128:(j + 1) * 128],
                                    wv_sb[:, kc, :],
                                    start=(kc == 0 and j == 0), stop=False,
                                    skip_group_check=True)
                    else:  # stage 7
                        v_ps = st.pop("v")
                        for kc in (6, 7):
                            for j in range(4):
                                nc.tensor.matmul(
                                    v_ps[:, j, :],
                                    xc[:, kc, j * 128:(j + 1) * 128],
                                    wv_sb[:, kc, :],
                                    start=False, stop=(kc == 7 and j == 3),
                                    skip_group_check=True)
                        tc0 = sc_i * 4
                        for j in range(4):
                            nc.vector.tensor_copy(
                                v_sb[:, tc0 + j, 0:dk], v_ps[:, j, 0:dk])
                            nc.vector.tensor_copy(
                                v_sb[:, tc0 + j, 3 * dk:4 * dk],
                                v_ps[:, j, dk:2 * dk])
                        del xc_tiles[sc_i]
                        pstate.pop(sc_i, None)

                # ---------------- batch 0 projections upfront ----------------
                for sc_i in range(c.nsb):
                    dma_x(sc_i)
                for sc_i in range(c.nsb):
                    for stage in range(8):
                        proj_stage(sc_i, stage)
                # prefetch x for the first interleaved proj chunk
                if c.b > 1:
                    dma_x(c.nsb)

                # ---------------- main loop ----------------
                # part2 is software-pipelined: the DVE reciprocal chain for
                # chunk n is emitted right after its t-loop, but the PE part
                # (bcast matmul, norm, out-proj) is deferred into chunk n+1's
                # first t-iteration so the PE streams scores while DVE works.
                pending = {}

                def part2_dve(b_i, sb, pva, pvb):
                    den_a = rpool.tile([1, c.sc], F32, tag="den")
                    den_b = rpool.tile([1, c.sc], F32, tag="den")
                    # NB: reciprocal_approx_fast directly on the PSUM rows
                    # returns garbage on HW even though an isolated probe of
                    # the same AP works — stage denominators through SBUF.
                    nc.vector.tensor_copy(den_a[:], pva[dk:dk + 1, :])
                    nc.vector.tensor_copy(den_b[:], pvb[0:1, :])
                    rec_af = rpool.tile([1, c.sc], F32, tag="recf")
                    rec_bf = rpool.tile([1, c.sc], F32, tag="recf")
                    nc.vector.reciprocal_approx_fast(out=rec_af[:], in_=den_a[:])
                    nc.vector.reciprocal_approx_fast(out=rec_bf[:], in_=den_b[:])
                    rec_ab = rpool.tile([1, c.sc], BF16, tag="recb")
                    rec_bb = rpool.tile([1, c.sc], BF16, tag="recb")
                    nc.vector.tensor_copy(rec_ab[:], rec_af[:])
                    nc.vector.tensor_copy(rec_bb[:], rec_bf[:])
                    pending.update(b_i=b_i, sb=sb, pva=pva, pvb=pvb,
                                   rec_ab=rec_ab, rec_bb=rec_bb)

                def part2_bc():
                    """Broadcast 1/denom to 64 partitions + normalize (nab)."""
                    if not pending:
                        return
                    b_i, sb = pending["b_i"], pending["sb"]
                    pva, pvb = pending["pva"], pending["pvb"]
                    bc2 = ps2.tile([128, 2 * c.sc], F32, tag="s2",
                                   name=f"bc2_{b_i}_{sb}")
                    nc.tensor.matmul(bc2[0:dk, 0:c.sc], ones64[:],
                                     pending["rec_ab"][:], start=True, stop=True)
                    nc.tensor.matmul(bc2[dk:2 * dk, c.sc:2 * c.sc], ones64[:],
                                     pending["rec_bb"][:], start=True, stop=True)
                    bcs = npool.tile([2 * dk, 2 * c.sc], F32, tag="bcs")
                    nc.scalar.copy(bcs[0:dk, 0:c.sc], bc2[0:dk, 0:c.sc])
                    nc.vector.tensor_copy(bcs[dk:2 * dk, c.sc:2 * c.sc],
                                          bc2[dk:2 * dk, c.sc:2 * c.sc])
                    nab = npool.tile([2 * dk, c.sc], BF16, tag="nab")
                    nc.vector.tensor_tensor(
                        nab[0:dk, :], pva[0:dk, :], bcs[0:dk, 0:c.sc],
                        ALU.mult)
                    nc.vector.tensor_tensor(
                        nab[dk:2 * dk, :], pvb[dk:2 * dk, :],
                        bcs[dk:2 * dk, c.sc:2 * c.sc], ALU.mult)
                    pending["nab"] = nab

                def part2_out():
                    if not pending:
                        return
                    b_i, sb = pending["b_i"], pending["sb"]
                    nab = pending["nab"]
                    s0 = b_i * c.s + sb * c.sc
                    for j in range(c.sc // 128):
                        o2 = ps2.tile([128, 2 * c.sc], F32, tag="s2",
                                      name=f"o2_{b_i}_{sb}_{j}")
                        for e in range(2):
                            nc.tensor.matmul(
                                o2[:, e * c.sc:(e + 1) * c.sc],
                                nab[:, j * 128:(j + 1) * 128],
                                wo_sb[:, e * c.sc:(e + 1) * c.sc],
                                start=True, stop=True)
                        osb = opool.tile([128, c.d], F32, tag="osb",
                                         name=f"osb_{b_i}_{sb}_{j}")
                        if j % 2 == 0:
                            nc.scalar.copy(osb[:], o2[:])
                        else:
                            nc.vector.tensor_copy(osb[:], o2[:])
                        nc.sync.dma_start(
                            out=out[s0 + j * 128:s0 + (j + 1) * 128, :],
                            in_=osb[:])
                    pending.clear()

                for b_i in range(c.b):
                    for sb in range(c.nsb):
                        s0 = b_i * c.s + sb * c.sc
                        proj_c = (b_i + 1) * c.nsb + sb if b_i + 1 < c.b else None
                        pva = pvp.tile([128, c.sc], F32, tag="pva")
                        pvb = pvp.tile([128, c.sc], F32, tag="pvb")
                        for tp in range(c.ntp):
                            t0 = b_i * c.s + (2 * tp) * 128
                            t1 = t0 + 128
                            s2a = ps2.tile([128, 2 * c.sc], F32, tag="s2",
                                           name=f"s2a_{b_i}_{sb}_{tp}")
                            nc.tensor.matmul(
                                s2a[:, 0:c.sc], kT_sb[0:dk, t0:t0 + 128],
                                qT_sb[0:dk, s0:s0 + c.sc],
                                start=True, stop=True)
                            nc.tensor.matmul(
                                s2a[:, c.sc:2 * c.sc], kT_sb[0:dk, t1:t1 + 128],
                                qT_sb[0:dk, s0:s0 + c.sc],
                                start=True, stop=True)
                            e2a = epool.tile([128, 2 * c.sc], BF16, tag="e2",
                                             name=f"e2a_{b_i}_{sb}_{tp}")
                            nc.scalar.activation(e2a[:], s2a[:], AF.Exp,
                                                 scale=1.0 / np.sqrt(dk))
                            s2b = ps2.tile([128, 2 * c.sc], F32, tag="s2",
                                           name=f"s2b_{b_i}_{sb}_{tp}")
                            nc.tensor.matmul(
                                s2b[:, 0:c.sc], kT_sb[dk:2 * dk, t0:t0 + 128],
                                qT_sb[dk:2 * dk, s0:s0 + c.sc],
                                start=True, stop=True)
                            nc.tensor.matmul(
                                s2b[:, c.sc:2 * c.sc], kT_sb[dk:2 * dk, t1:t1 + 128],
                                qT_sb[dk:2 * dk, s0:s0 + c.sc],
                                start=True, stop=True)
                            e2b = epool.tile([128, 2 * c.sc], BF16, tag="e2",
                                             name=f"e2b_{b_i}_{sb}_{tp}")
                            nc.scalar.activation(e2b[:], s2b[:], AF.Exp,
                                                 scale=1.0 / np.sqrt(dk))
                            if tp == 0:
                                part2_bc()  # deferred from the previous chunk
                            elif tp == 1:
                                part2_out()
                            # proj filler here gives the exps time to land
                            # before the pv matmuls consume them
                            if proj_c is not None:
                                for st_i in range(tp * 8 // c.ntp,
                                                  (tp + 1) * 8 // c.ntp):
                                    proj_stage(proj_c, st_i)
                            tca = b_i * c.ntb + 2 * tp
                            tcb = tca + 1
                            nc.tensor.matmul(
                                pva[:], v_sb[:, tca, 0:2 * dk], e2a[:, 0:c.sc],
                                start=(tp == 0), stop=False)
                            nc.tensor.matmul(
                                pva[:], v_sb[:, tcb, 0:2 * dk],
                                e2a[:, c.sc:2 * c.sc],
                                start=False, stop=(tp == c.ntp - 1))
                            nc.tensor.matmul(
                                pvb[:], v_sb[:, tca, 2 * dk:4 * dk],
                                e2b[:, 0:c.sc],
                                start=(tp == 0), stop=False)
                            nc.tensor.matmul(
                                pvb[:], v_sb[:, tcb, 2 * dk:4 * dk],
                                e2b[:, c.sc:2 * c.sc],
                                start=False, stop=(tp == c.ntp - 1))

                        part2_dve(b_i, sb, pva, pvb)

                        # prefetch x for the next interleaved proj chunk
                        if proj_c is not None:
                            nxt = proj_c + 1
                            if nxt < c.nsc and nxt not in xc_tiles:
                                dma_x(nxt)

                part2_bc()  # flush the final chunk
                part2_out()

    nc.compile()
    return nc


_NC_CACHE = {}


def get_nc(cfg: Cfg | None = None):
    cfg = cfg or Cfg()
    key = (cfg.b, cfg.s, cfg.d, cfg.cpc, cfg.dk)
    if key not in _NC_CACHE:
        _NC_CACHE[key] = _build_nc(cfg)
    return _NC_CACHE[key]


def make_in_maps(x, w_q, b_q, w_k, b_k, w_v, b_v, w_o, b_o,
                 a_q, u_q, a_k, u_k, a_v, u_v, cfg: Cfg | None = None):
    """Host-side prep: merge LoRA, transpose x, cast to bf16, shard."""
    c = cfg or Cfg()
    x = np.asarray(x, np.float32)
    w_o = np.asarray(w_o, np.float32)

    def merge(w, a, u):
        return (np.asarray(w, np.float64)
                + (np.asarray(a, np.float64) @ np.asarray(u, np.float64))
                * SCALING).astype(np.float32)

    wq_eff = merge(w_q, a_q, u_q)
    wk_eff = merge(w_k, a_k, u_k)
    wv_eff = merge(w_v, a_v, u_v)

    xT = np.ascontiguousarray(x.reshape(c.seq, c.d).T).astype(NP_BF16)
    b_q = np.asarray(b_q, np.float32)
    b_k = np.asarray(b_k, np.float32)
    in_maps = []
    for i in range(N_CORES):
        sl = slice(i * c.cpc, (i + 1) * c.cpc)
        in_maps.append({
            "xT": xT,
            "wq": np.ascontiguousarray(wq_eff[:, sl]).astype(NP_BF16),
            "wk": np.ascontiguousarray(wk_eff[:, sl]).astype(NP_BF16),
            "wv": np.ascontiguousarray(wv_eff[:, sl]).astype(NP_BF16),
            "wo": np.ascontiguousarray(w_o[sl, :]).astype(NP_BF16),
            "bq": np.ascontiguousarray(b_q[sl]).reshape(c.cpc, 1),
            "bk": np.ascontiguousarray(b_k[sl]).reshape(c.cpc, 1),
        })
    return in_maps


def kernel(x, w_q, b_q, w_k, b_k, w_v, b_v, w_o, b_o,
           a_q, u_q, a_k, u_k, a_v, u_v):
    cfg = Cfg()
    c = cfg
    in_maps = make_in_maps(x, w_q, b_q, w_k, b_k, w_v, b_v, w_o, b_o,
                           a_q, u_q, a_k, u_k, a_v, u_v, cfg)
    nc = get_nc(cfg)
    res = run_bass_kernel_spmd(nc, in_maps, list(range(N_CORES)))
    out = np.zeros((c.seq, c.d), np.float32)
    for i in range(N_CORES):
        out += res.results[i]["out"]
    # v-bias rides through softmax as a constant row; b_o is plain bias
    b_v = np.asarray(b_v, np.float32)
    b_o = np.asarray(b_o, np.float32)
    w_o = np.asarray(w_o, np.float32)
    out += (b_v @ w_o + b_o).astype(np.float32)
    return out.reshape(B, S, D_MODEL).astype(np.float32)


# revision 28
# speedup vs baseline: 2.4592x; 1.0620x over previous
"""LoRA attention Bass kernel for 8x Trainium2 NeuronCores.

Sharding (Megatron tensor-parallel over heads):
  - Each of the 8 cores owns 2 heads (128 projection columns).
  - q/k/v projections column-sharded; out projection row-sharded;
    per-core partial outputs are summed on the host.
  - LoRA is merged into the base weights on the host (w_eff = w + a@u*scaling),
    which is exact up to fp32 rounding.

All matmul operands are bf16 (fp32/f32r matmuls trip the PE power throttle
to a 50% duty cycle; bf16 streams at 1 row/cycle at 2.4 GHz). PSUM
accumulation stays fp32.

Device schedule (per core):
  Phase 1 (proj): qT/kT computed transposed ([proj_col, seq]) from xT tiles;
      v computed in natural layout ([seq, proj_col]). Biases folded in via
      DVE tensor_scalar on the PSUM->SBUF copy. Projections for batch b+1 are
      interleaved into batch b's attention t-loop to fill PE idle slots
      (the t-loop is ACT-bound).
  Phase 2 (attention): S^T = K @ Q^T per (batch, head); scores for two
      t-chunks share one 2-bank PSUM tile so a single ACT exp instruction
      covers 1024 elements (halves ACT per-instruction overhead). P@V uses
      lhsT=[v | ones] so the softmax denominator falls out of the same
      matmul (row 64 of the PSUM output).
  Phase 3 (out-proj): recip via DVE reciprocal_approx_fast, broadcast to 64
      partitions via a K=1 ones matmul, normalize+cast to bf16 on DVE, then
      out = attnout @ Wo_slice accumulated in PSUM and DMA'd out.

PSUM budget (8 banks): s2 tag 2x[128,1024] (scores / bcast / out-proj) = 4,
  pv_a + pv_b = 2, proj accumulators (q/k/v rotating, bufs=2) = 2.
"""

import numpy as np
import ml_dtypes

import concourse.bass as bass
import concourse.mybir as mybir
import concourse.tile as tile
from concourse import bacc
from concourse.bass_utils import run_bass_kernel_spmd

F32 = mybir.dt.float32
F32R = mybir.dt.float32r
BF16 = mybir.dt.bfloat16
AF = mybir.ActivationFunctionType
ALU = mybir.AluOpType

N_CORES = 8

# Full-problem dims (hardcoded per spec)
D_MODEL = 1024
N_HEADS = 16
D_K = 64
LORA_R = 8
SCALING = 2.0
B = 4
S = 2048

NP_BF16 = ml_dtypes.bfloat16


class Cfg:
    def __init__(self, b=B, s=S, d=D_MODEL, cpc=128, dk=D_K):
        self.b = b                      # batches
        self.s = s                      # seq per batch
        self.d = d                      # model dim
        self.cpc = cpc                  # projection cols per core (2 heads x 64)
        self.dk = dk                    # head dim
        self.seq = b * s                # total rows
        self.nkc = d // 128             # k chunks for projections
        self.sc = 512                   # s-chunk width (free dim of matmuls)
        self.nsc = self.seq // self.sc  # s chunks over the whole input
        self.ntb = s // 128             # t chunks per batch
        self.ntp = self.ntb // 2        # t-chunk pairs per batch
        self.nsb = s // self.sc         # s chunks per batch
        self.ntc_g = self.seq // 128    # global t chunks


def _build_nc(cfg: Cfg):
    c = cfg
    dk = c.dk
    nc = bacc.Bacc("TRN2", target_bir_lowering=False, debug=False,
                   num_devices=N_CORES)

    xT = nc.dram_tensor("xT", [c.d, c.seq], BF16, kind="ExternalInput").ap()
    wq = nc.dram_tensor("wq", [c.d, c.cpc], BF16, kind="ExternalInput").ap()
    wk = nc.dram_tensor("wk", [c.d, c.cpc], BF16, kind="ExternalInput").ap()
    wv = nc.dram_tensor("wv", [c.d, c.cpc], BF16, kind="ExternalInput").ap()
    wo = nc.dram_tensor("wo", [c.cpc, c.d], BF16, kind="ExternalInput").ap()
    bq = nc.dram_tensor("bq", [c.cpc, 1], F32, kind="ExternalInput").ap()
    bk = nc.dram_tensor("bk", [c.cpc, 1], F32, kind="ExternalInput").ap()
    out = nc.dram_tensor("out", [c.seq, c.d], F32, kind="ExternalOutput").ap()

    xT_r = xT.rearrange("(kc p) s -> p kc s", p=128)

    with tile.TileContext(nc) as tc:
        with tc.tile_pool(name="persist", bufs=1) as persist:
            qT_sb = persist.tile([128, c.seq], BF16, tag="qT")
            kT_sb = persist.tile([128, c.seq], BF16, tag="kT")
            # v in PV-lhsT layout, 128 cols per head per t-chunk:
            #   cols 0:64    = vA          (PV-A out partitions 0:64 = attnA)
            #   col 64       = ones        (PV-A out partition 64 = denomA)
            #   cols 65:128  = zeros
            #   col 128      = ones        (PV-B out partition 0 = denomB)
            #   cols 129:192 = zeros
            #   cols 192:256 = vB          (PV-B out partitions 64:128 = attnB)
            # so attnA lands at psum partitions 0:64 and attnB at 64:128,
            # letting the out-projection contract both heads in one K=128
            # matmul against the unsplit wo.
            v_sb = persist.tile([128, c.ntc_g, 4 * dk], BF16, tag="v")
            wq_sb = persist.tile([128, c.nkc, c.cpc], BF16, tag="wq")
            wk_sb = persist.tile([128, c.nkc, c.cpc], BF16, tag="wk")
            wv_sb = persist.tile([128, c.nkc, c.cpc], BF16, tag="wv")
            wo_sb = persist.tile([c.cpc, c.d], BF16, tag="wo")
            bq_sb = persist.tile([c.cpc, 1], F32, tag="bq")
            bk_sb = persist.tile([c.cpc, 1], F32, tag="bk")
            ones64 = persist.tile([1, dk], BF16, tag="ones64")

            # DMA order matters at startup: the first q-projection only needs
            # x chunk 0 (issued in the main block below) + wq + bq.
            nc.sync.dma_start(out=wq_sb[:], in_=wq.rearrange("(kc p) m -> p kc m", p=128))
            nc.sync.dma_start(out=bq_sb[:], in_=bq[:])
            nc.sync.dma_start(out=wk_sb[:], in_=wk.rearrange("(kc p) m -> p kc m", p=128))
            nc.sync.dma_start(out=bk_sb[:], in_=bk[:])
            nc.sync.dma_start(out=wv_sb[:], in_=wv.rearrange("(kc p) m -> p kc m", p=128))
            nc.sync.dma_start(out=wo_sb[:], in_=wo[:])

            ones_f32 = persist.tile([128, 1], F32, tag="ones_f32")
            nc.vector.memset(v_sb[:], 0.0)
            nc.vector.memset(ones_f32[:], 1.0)
            nc.vector.tensor_copy(ones64[:], ones_f32[0:1, :].to_broadcast([1, dk]))
            nc.vector.tensor_copy(
                v_sb[:, :, dk:dk + 1],
                ones_f32[:].unsqueeze(1).to_broadcast([128, c.ntc_g, 1]))
            nc.vector.tensor_copy(
                v_sb[:, :, 2 * dk:2 * dk + 1],
                ones_f32[:].unsqueeze(1).to_broadcast([128, c.ntc_g, 1]))

            with tc.tile_pool(name="xin", bufs=5) as xpool, \
                 tc.tile_pool(name="ps2", bufs=2, space="PSUM") as ps2, \
                 tc.tile_pool(name="pvp", bufs=1, space="PSUM") as pvp, \
                 tc.tile_pool(name="prj", bufs=2, space="PSUM") as prj, \
                 tc.tile_pool(name="exp", bufs=4) as epool, \
                 tc.tile_pool(name="norm", bufs=2) as npool, \
                 tc.tile_pool(name="rec", bufs=2) as rpool, \
                 tc.tile_pool(name="osb", bufs=3) as opool:

                xc_tiles = {}

                def dma_x(sc_i):
                    x_t = xpool.tile([128, c.nkc, c.sc], BF16, tag="x",
                                     name=f"xc_{sc_i}")
                    s0 = sc_i * c.sc
                    nc.sync.dma_start(out=x_t[:], in_=xT_r[:, :, s0:s0 + c.sc])
                    xc_tiles[sc_i] = x_t

                # Per-chunk projection state (psum tiles held across stages)
                pstate = {}

                def proj_stage(sc_i, stage):
                    """Emit 1/8th of projection chunk sc_i (stages 0..7)."""
                    xc = xc_tiles[sc_i]
                    s0 = sc_i * c.sc
                    st = pstate.setdefault(sc_i, {})
                    if stage == 0:
                        q_ps = prj.tile([128, c.sc], F32, tag="prj",
                                        name=f"q_ps_{sc_i}")
                        st["q"] = q_ps
                        for kc in range(4):
                            nc.tensor.matmul(q_ps[:], wq_sb[:, kc, :],
                                             xc[:, kc, :],
                                             start=(kc == 0), stop=False)
                    elif stage == 1:
                        q_ps = st.pop("q")
                        for kc in range(4, c.nkc):
                            nc.tensor.matmul(q_ps[:], wq_sb[:, kc, :],
                                             xc[:, kc, :],
                                             start=False, stop=(kc == c.nkc - 1))
                        nc.vector.tensor_scalar(
                            qT_sb[:, s0:s0 + c.sc], q_ps[:], bq_sb[:], None,
                            ALU.add)
                    elif stage == 2:
                        k_ps = prj.tile([128, c.sc], F32, tag="prj",
                                        name=f"k_ps_{sc_i}")
                        st["k"] = k_ps
                        for kc in range(4):
                            nc.tensor.matmul(k_ps[:], wk_sb[:, kc, :],
                                             xc[:, kc, :],
                                             start=(kc == 0), stop=False)
                    elif stage == 3:
                        k_ps = st.pop("k")
                        for kc in range(4, c.nkc):
                            nc.tensor.matmul(k_ps[:], wk_sb[:, kc, :],
                                             xc[:, kc, :],
                                             start=False, stop=(kc == c.nkc - 1))
                        nc.vector.tensor_scalar(
                            kT_sb[:, s0:s0 + c.sc], k_ps[:], bk_sb[:], None,
                            ALU.add)
                    elif stage in (4, 5, 6):
                        if stage == 4:
                            v_ps = prj.tile([128, 4, 128], F32, tag="prj",
                                            name=f"v_ps_{sc_i}")
                            st["v"] = v_ps
                        v_ps = st["v"]
                        # One accumulation group for the whole bank: start
                        # zeroes the full 2KB zero region, so only the very
                        # first matmul may set start and only the last stop.
                        for kc in range(2 * (stage - 4), 2 * (stage - 4) + 2):
                            for j in range(4):
                                nc.tensor.matmul(
                                    v_ps[:, j, :],
                                    xc[:, kc, j * 128:(j + 1) * 128],
                                    wv_sb[:, kc, :],
                                    start=(kc == 0 and j == 0), stop=False,
                                    skip_group_check=True)
                    else:  # stage 7
                        v_ps = st.pop("v")
                        for kc in (6, 7):
                            for j in range(4):
                                nc.tensor.matmul(
                                    v_ps[:, j, :],
                                    xc[:, kc, j * 128:(j + 1) * 128],
                                    wv_sb[:, kc, :],
                                    start=False, stop=(kc == 7 and j == 3),
                                    skip_group_check=True)
                        tc0 = sc_i * 4
                        for j in range(4):
                            nc.vector.tensor_copy(
                                v_sb[:, tc0 + j, 0:dk], v_ps[:, j, 0:dk])
                            nc.vector.tensor_copy(
                                v_sb[:, tc0 + j, 3 * dk:4 * dk],
                                v_ps[:, j, dk:2 * dk])
                        del xc_tiles[sc_i]
                        pstate.pop(sc_i, None)

                # ---------------- batch 0 projections upfront ----------------
                # q for all chunks first, then k, then v — matches the
                # weight-DMA arrival order so the PE starts ASAP.
                for sc_i in range(c.nsb):
                    dma_x(sc_i)
                for st0 in (0, 2, 4):
                    for sc_i in range(c.nsb):
                        for stage in range(st0, st0 + 2 if st0 < 4 else 8):
                            proj_stage(sc_i, stage)
                # prefetch x for the first interleaved proj chunk
                if c.b > 1:
                    dma_x(c.nsb)

                # ---------------- main loop ----------------
                # part2 is software-pipelined: the DVE reciprocal chain for
                # chunk n is emitted right after its t-loop, but the PE part
                # (bcast matmul, norm, out-proj) is deferred into chunk n+1's
                # first t-iteration so the PE streams scores while DVE works.
                pending = {}

                def part2_dve(b_i, sb, pva, pvb):
                    den_a = rpool.tile([1, c.sc], F32, tag="den")
                    den_b = rpool.tile([1, c.sc], F32, tag="den")
                    # NB: reciprocal_approx_fast directly on the PSUM rows
                    # returns garbage on HW even though an isolated probe of
                    # the same AP works — stage denominators through SBUF.
                    nc.vector.tensor_copy(den_a[:], pva[dk:dk + 1, :])
                    nc.vector.tensor_copy(den_b[:], pvb[0:1, :])
                    rec_af = rpool.tile([1, c.sc], F32, tag="recf")
                    rec_bf = rpool.tile([1, c.sc], F32, tag="recf")
                    nc.vector.reciprocal_approx_fast(out=rec_af[:], in_=den_a[:])
                    nc.vector.reciprocal_approx_fast(out=rec_bf[:], in_=den_b[:])
                    rec_ab = rpool.tile([1, c.sc], BF16, tag="recb")
                    rec_bb = rpool.tile([1, c.sc], BF16, tag="recb")
                    nc.vector.tensor_copy(rec_ab[:], rec_af[:])
                    nc.vector.tensor_copy(rec_bb[:], rec_bf[:])
                    pending.update(b_i=b_i, sb=sb, pva=pva, pvb=pvb,
                                   rec_ab=rec_ab, rec_bb=rec_bb)

                def part2_bc():
                    """Broadcast 1/denom to 64 partitions + normalize (nab)."""
                    if not pending:
                        return
                    b_i, sb = pending["b_i"], pending["sb"]
                    pva, pvb = pending["pva"], pending["pvb"]
                    bc_a = prj.tile([dk, c.sc], F32, tag="prj",
                                    name=f"bca_{b_i}_{sb}")
                    bc_b = prj.tile([2 * dk, c.sc], F32, tag="prj",
                                    name=f"bcb_{b_i}_{sb}")
                    nc.tensor.matmul(bc_a[:], ones64[:],
                                     pending["rec_ab"][:], start=True, stop=True)
                    nc.tensor.matmul(bc_b[dk:2 * dk, :], ones64[:],
                                     pending["rec_bb"][:], start=True, stop=True)
                    bcs = npool.tile([2 * dk, 2 * c.sc], F32, tag="bcs")
                    nc.scalar.copy(bcs[0:dk, 0:c.sc], bc_a[:])
                    nc.vector.tensor_copy(bcs[dk:2 * dk, c.sc:2 * c.sc],
                                          bc_b[dk:2 * dk, :])
                    nab = npool.tile([2 * dk, c.sc], BF16, tag="nab")
                    nc.vector.tensor_tensor(
                        nab[0:dk, :], pva[0:dk, :], bcs[0:dk, 0:c.sc],
                        ALU.mult)
                    nc.vector.tensor_tensor(
                        nab[dk:2 * dk, :], pvb[dk:2 * dk, :],
                        bcs[dk:2 * dk, c.sc:2 * c.sc], ALU.mult)
                    pending["nab"] = nab

                def part2_out():
                    if not pending:
                        return
                    b_i, sb = pending["b_i"], pending["sb"]
                    nab = pending["nab"]
                    s0 = b_i * c.s + sb * c.sc
                    for j in range(c.sc // 128):
                        o2 = ps2.tile([128, 2 * c.sc], F32, tag="s2",
                                      name=f"o2_{b_i}_{sb}_{j}")
                        for e in range(2):
                            nc.tensor.matmul(
                                o2[:, e * c.sc:(e + 1) * c.sc],
                                nab[:, j * 128:(j + 1) * 128],
                                wo_sb[:, e * c.sc:(e + 1) * c.sc],
                                start=True, stop=True)
                        osb = opool.tile([128, c.d], F32, tag="osb",
                                         name=f"osb_{b_i}_{sb}_{j}")
                        if j % 2 == 0:
                            nc.scalar.copy(osb[:], o2[:])
                        else:
                            nc.vector.tensor_copy(osb[:], o2[:])
                        nc.sync.dma_start(
                            out=out[s0 + j * 128:s0 + (j + 1) * 128, :],
                            in_=osb[:])
                    pending.clear()

                for b_i in range(c.b):
                    for sb in range(c.nsb):
                        s0 = b_i * c.s + sb * c.sc
                        proj_c = (b_i + 1) * c.nsb + sb if b_i + 1 < c.b else None
                        pva = pvp.tile([128, c.sc], F32, tag="pva")
                        pvb = pvp.tile([128, c.sc], F32, tag="pvb")
                        for tp in range(c.ntp):
                            t0 = b_i * c.s + (2 * tp) * 128
                            t1 = t0 + 128
                            s2a = ps2.tile([128, 2 * c.sc], F32, tag="s2",
                                           name=f"s2a_{b_i}_{sb}_{tp}")
                            nc.tensor.matmul(
                                s2a[:, 0:c.sc], kT_sb[0:dk, t0:t0 + 128],
                                qT_sb[0:dk, s0:s0 + c.sc],
                                start=True, stop=True)
                            nc.tensor.matmul(
                                s2a[:, c.sc:2 * c.sc], kT_sb[0:dk, t1:t1 + 128],
                                qT_sb[0:dk, s0:s0 + c.sc],
                                start=True, stop=True)
                            e2a = epool.tile([128, 2 * c.sc], BF16, tag="e2",
                                             name=f"e2a_{b_i}_{sb}_{tp}")
                            nc.scalar.activation(e2a[:], s2a[:], AF.Exp,
                                                 scale=1.0 / np.sqrt(dk))
                            s2b = ps2.tile([128, 2 * c.sc], F32, tag="s2",
                                           name=f"s2b_{b_i}_{sb}_{tp}")
                            nc.tensor.matmul(
                                s2b[:, 0:c.sc], kT_sb[dk:2 * dk, t0:t0 + 128],
                                qT_sb[dk:2 * dk, s0:s0 + c.sc],
                                start=True, stop=True)
                            nc.tensor.matmul(
                                s2b[:, c.sc:2 * c.sc], kT_sb[dk:2 * dk, t1:t1 + 128],
                                qT_sb[dk:2 * dk, s0:s0 + c.sc],
                                start=True, stop=True)
                            e2b = epool.tile([128, 2 * c.sc], BF16, tag="e2",
                                             name=f"e2b_{b_i}_{sb}_{tp}")
                            nc.scalar.activation(e2b[:], s2b[:], AF.Exp,
                                                 scale=1.0 / np.sqrt(dk))
                            if tp == 0:
                                part2_bc()  # deferred from the previous chunk
                            elif tp == 1:
                                part2_out()
                            # proj filler here gives the exps time to land
                            # before the pv matmuls consume them
                            if proj_c is not None:
                                for st_i in range(tp * 8 // c.ntp,
                                                  (tp + 1) * 8 // c.ntp):
                                    proj_stage(proj_c, st_i)
                            tca = b_i * c.ntb + 2 * tp
                            tcb = tca + 1
                            nc.tensor.matmul(
                                pva[:], v_sb[:, tca, 0:2 * dk], e2a[:, 0:c.sc],
                                start=(tp == 0), stop=False)
                            nc.tensor.matmul(
                                pva[:], v_sb[:, tcb, 0:2 * dk],
                                e2a[:, c.sc:2 * c.sc],
                                start=False, stop=(tp == c.ntp - 1))
                            nc.tensor.matmul(
                                pvb[:], v_sb[:, tca, 2 * dk:4 * dk],
                                e2b[:, 0:c.sc],
                                start=(tp == 0), stop=False)
                            nc.tensor.matmul(
                                pvb[:], v_sb[:, tcb, 2 * dk:4 * dk],
                                e2b[:, c.sc:2 * c.sc],
                                start=False, stop=(tp == c.ntp - 1))

                        part2_dve(b_i, sb, pva, pvb)

                        # prefetch x for the next interleaved proj chunk
                        if proj_c is not None:
                            nxt = proj_c + 1
                            if nxt < c.nsc and nxt not in xc_tiles:
                                dma_x(nxt)

                part2_bc()  # flush the final chunk
                part2_out()

    nc.compile()
    return nc


_NC_CACHE = {}


def get_nc(cfg: Cfg | None = None):
    cfg = cfg or Cfg()
    key = (cfg.b, cfg.s, cfg.d, cfg.cpc, cfg.dk)
    if key not in _NC_CACHE:
        _NC_CACHE[key] = _build_nc(cfg)
    return _NC_CACHE[key]


def make_in_maps(x, w_q, b_q, w_k, b_k, w_v, b_v, w_o, b_o,
                 a_q, u_q, a_k, u_k, a_v, u_v, cfg: Cfg | None = None):
    """Host-side prep: merge LoRA, transpose x, cast to bf16, shard."""
    c = cfg or Cfg()
    x = np.asarray(x, np.float32)
    w_o = np.asarray(w_o, np.float32)

    def merge(w, a, u):
        return (np.asarray(w, np.float64)
                + (np.asarray(a, np.float64) @ np.asarray(u, np.float64))
                * SCALING).astype(np.float32)

    wq_eff = merge(w_q, a_q, u_q)
    wk_eff = merge(w_k, a_k, u_k)
    wv_eff = merge(w_v, a_v, u_v)

    xT = np.ascontiguousarray(x.reshape(c.seq, c.d).T).astype(NP_BF16)
    b_q = np.asarray(b_q, np.float32)
    b_k = np.asarray(b_k, np.float32)
    in_maps = []
    for i in range(N_CORES):
        sl = slice(i * c.cpc, (i + 1) * c.cpc)
        in_maps.append({
            "xT": xT,
            "wq": np.ascontiguousarray(wq_eff[:, sl]).astype(NP_BF16),
            "wk": np.ascontiguousarray(wk_eff[:, sl]).astype(NP_BF16),
            "wv": np.ascontiguousarray(wv_eff[:, sl]).astype(NP_BF16),
            "wo": np.ascontiguousarray(w_o[sl, :]).astype(NP_BF16),
            "bq": np.ascontiguousarray(b_q[sl]).reshape(c.cpc, 1),
            "bk": np.ascontiguousarray(b_k[sl]).reshape(c.cpc, 1),
        })
    return in_maps


def kernel(x, w_q, b_q, w_k, b_k, w_v, b_v, w_o, b_o,
           a_q, u_q, a_k, u_k, a_v, u_v):
    cfg = Cfg()
    c = cfg
    in_maps = make_in_maps(x, w_q, b_q, w_k, b_k, w_v, b_v, w_o, b_o,
                           a_q, u_q, a_k, u_k, a_v, u_v, cfg)
    nc = get_nc(cfg)
    res = run_bass_kernel_spmd(nc, in_maps, list(range(N_CORES)))
    out = np.zeros((c.seq, c.d), np.float32)
    for i in range(N_CORES):
        out += res.results[i]["out"]
    # v-bias rides through softmax as a constant row; b_o is plain bias
    b_v = np.asarray(b_v, np.float32)
    b_o = np.asarray(b_o, np.float32)
    w_o = np.asarray(w_o, np.float32)
    out += (b_v @ w_o + b_o).astype(np.float32)
    return out.reshape(B, S, D_MODEL).astype(np.float32)
